# revision 33
# baseline (speedup 1.0000x reference)
"""Trainium2 8-core Bass kernel for nn_Decoder (single-step LSTM decoder with
Gaussian-windowed attention and a 32k-vocab log-softmax head).

Sharding strategy (tensor-parallel over all heavy weights):
  - LSTM: hidden units sharded 128/core; gate rows [i,f,g,o] for this core's
    units gathered into a (2048, 512) transposed weight block per layer.
    AllGather (128 floats/core) of h after each layer.
  - Attention: p-network replicated (tiny); encoder positions sharded
    512/core, window mask computed densely; AllReduce of
    [ctx_partial(1024), Z_partial(1)].
  - fc1: output units sharded 128/core, AllGather of `out`.
  - fc2/log_softmax: vocab padded to 32768 and sharded 4096/core; the
    softmax denominator is AllGathered (8 scalars) and summed locally.

All matvecs run on the TensorEngine with host-pre-transposed weights so the
contraction dim lands on SBUF partitions with fully contiguous DMA lines.
"""

import numpy as np
from ml_dtypes import bfloat16
import concourse.bass as bass
import concourse.mybir as mybir
from concourse import tile
from concourse.bass_utils import run_bass_kernel_spmd

NCORE = 8
H = 1024
L = 2
V = 32000
VP = 32768          # vocab padded to 8*4096
VS = VP // NCORE    # 4096 per core
S = 4096
WW = 64
WIN = 2 * WW + 1
STD2 = 2.0 * (WW / 2.0) ** 2   # 2048
NEG = -1e30
F32 = mybir.dt.float32
BF16 = mybir.dt.bfloat16
AF = mybir.ActivationFunctionType
ALU = mybir.AluOpType
RG = [list(range(NCORE))]


# --------------------------------------------------------------------------
# Workaround: the walrus build in this container rejects instructions with
# more than ONE sync-wait command. Waits are AND-conditions evaluated on an
# in-order engine queue, so excess waits are moved onto NoOps inserted
# immediately before the instruction.
def _split_sync_waits(nc, max_waits=1):
    for fn in nc.m.functions:
        for blk in fn.blocks:
            instrs = list(blk.instructions)
            new_instrs = []
            changed = False
            for ins in instrs:
                si = ins.sync_info
                waits = list(si.on_wait) if si is not None else []
                if len(waits) > max_waits:
                    extra = waits[:-max_waits]
                    keep = waits[-max_waits:]
                    for j, w in enumerate(extra):
                        nop = mybir.InstNoOp(
                            name=f"{ins.name}-wsplit{j}", ins=[], outs=[],
                            sync_info=mybir.SyncInfo(on_wait=[w], on_update=[]),
                        )
                        nop.engine = ins.engine
                        new_instrs.append(nop)
                    ins.sync_info = mybir.SyncInfo(
                        on_wait=keep, on_update=list(si.on_update))
                    changed = True
                new_instrs.append(ins)
            if changed:
                blk.instructions = new_instrs


# --------------------------------------------------------------------------
def _build(include_hh: bool):
    nc = bass.Bass(num_devices=NCORE)
    KZ = 16 if include_hh else 8        # contraction k-tiles per LSTM layer
    WCROWS = 128 * KZ

    def inp(name, shape):
        return nc.dram_tensor(name, shape, F32, kind="ExternalInput")

    xcols = inp("xcols", [128, 8])
    h0cols = inp("h0cols", [128, 16]) if include_hh else None
    c0row = inp("c0row", [1, 128])
    wcat0_h = inp("wcat0", [WCROWS, 512])
    wcat1_h = inp("wcat1", [128 * (2 if include_hh else 1), 4096])
    bias2i_h = inp("bias2i", [128, 32])
    bias2h_h = inp("bias2h", [128, 32])
    c02c_h = inp("c02c", [128, 8])
    oneh_h = inp("oneh", [1, 8])
    h0sh_h = inp("h0sh", [128, 1]) if include_hh else None
    biasg = inp("biasg", [1, 1024])
    attw = inp("attw", [1024, 512])
    attb_col = inp("attb_col", [128, 4])
    attbr_h = inp("attbr", [1, 512])
    attw2col = inp("attw2col", [128, 4])
    attb2 = inp("attb2", [1, 1])
    sconst = inp("sconst", [1, 2])
    iota4 = inp("iota4", [128, 4])
    iotas = inp("iotas", [128, 4])
    encT = inp("encT", [1024, 512])
    encN = inp("encN", [512, 1024])
    fc1w = inp("fc1w", [2048, 128])
    fc1b_col = inp("fc1b_col", [128, 1])
    fc2w = nc.dram_tensor("fc2w", [128, VP], BF16, kind="ExternalInput")
    fc2b = inp("fc2b", [128, VP // 128])
    onesr = inp("onesr", [1, 128])
    onesc = inp("onesc", [128, 1])
    zpad = inp("zpad", [1, 8])

    y_part = nc.dram_tensor("y_part", [128, VP // 128], F32, kind="ExternalOutput")
    out_part = nc.dram_tensor("out_part", [128, 1], F32, kind="ExternalOutput")
    hc1blk = nc.dram_tensor("hc1blk", [2048, 1], F32, kind="ExternalOutput")
    h2cols = nc.dram_tensor("h2cols", [128, 8], F32, kind="ExternalOutput")
    c2cols = nc.dram_tensor("c2cols", [128, 8], F32, kind="ExternalOutput")
    a_part = nc.dram_tensor("a_part", [128, 4], F32, kind="ExternalOutput")
    aux = nc.dram_tensor("aux", [1, 4], F32, kind="ExternalOutput")

    with tile.TileContext(nc) as tc:
        with (
            tc.tile_pool(name="smalls", bufs=1) as sp,
            tc.tile_pool(name="wcatp", bufs=3) as wcatp,
            tc.tile_pool(name="attp", bufs=2) as attp,
            tc.tile_pool(name="encTp", bufs=2) as encTp,
            tc.tile_pool(name="encNp", bufs=2) as encNp,
            tc.tile_pool(name="fc1p", bufs=2) as fc1p,
            tc.tile_pool(name="fc2p", bufs=8) as fc2p,
            tc.tile_pool(name="dram", bufs=1, space="DRAM") as dp,
        ):
            # ---------- collective warm-up (absorbs first-CC staging) ----
            zpd = sp.tile([1, 8], F32, tag="zpd")
            nc.sync.dma_start(zpd[:], zpad[:, :])
            warm_in = dp.tile([6144, 1], F32, tag="warm_in")
            warm_out = dp.tile([6144, 1], F32, tag="warm_out")
            # no input dependency: fires at t=0, content irrelevant
            nc.gpsimd.collective_compute(
                "AllReduce", ALU.add, replica_groups=RG,
                ins=[warm_in.opt()], outs=[warm_out.opt()])

            # ---------- tiny LSTM activations first (matmul lhsT inputs) --
            xc = sp.tile([128, 8], F32, tag="xc")
            nc.sync.dma_start(xc[:], xcols[:, :])
            if include_hh:
                h0c = sp.tile([128, 16], F32, tag="h0c")
                nc.sync.dma_start(h0c[:], h0cols[:, :])

            # ---------- LSTM weights stream first (critical path head) ----
            # layer 0 in small (128,1024) chunks so the first matmul can
            # start as early as possible; layer 1 in (128,2048) chunks.
            wcchunks0 = []
            for a in range(KZ // 2):
                wc = wcatp.tile([128, 1024], F32, tag="wcat0",
                                name=f"wcat0_{a}", bufs=4 if include_hh else KZ // 2)
                nc.sync.dma_start(
                    wc[:].rearrange("p (j n) -> p j n", j=2),
                    wcat0_h[256 * a:256 * (a + 1), :].rearrange(
                        "(j p) n -> p j n", p=128))
                wcchunks0.append(wc)
            # layer-2 weights: own 128 contraction rows x all 4096 gates
            w2sb = []
            for r in range(2):
                wt = wcatp.tile([128, 2048], F32, tag="wcat",
                                name=f"w2sb{r}", bufs=2)
                nc.sync.dma_start(wt[:], wcat1_h[0:128, 2048 * r:2048 * (r + 1)])
                w2sb.append(wt)
            if include_hh:
                w2hb = []
                for r in range(2):
                    wt = wcatp.tile([128, 2048], F32, tag="wcath",
                                    name=f"w2hb{r}", bufs=2)
                    nc.sync.dma_start(wt[:], wcat1_h[128:256, 2048 * r:2048 * (r + 1)])
                    w2hb.append(wt)
                h0shs = sp.tile([128, 1], F32, tag="h0shs")
                nc.sync.dma_start(h0shs[:], h0sh_h[:, :])

            # ---------- small resident inputs ----
            c0s = sp.tile([1, 128], F32, tag="c0s")
            nc.sync.dma_start(c0s[:], c0row[:, :])
            bgs = sp.tile([1, 1024], F32, tag="bgs")
            nc.sync.dma_start(bgs[:], biasg[:, :])
            bsum = sp.tile([1, 512], F32, tag="bsum")
            nc.vector.tensor_tensor(bsum[:, :], bgs[0:1, 0:512], bgs[0:1, 512:1024], ALU.add)
            b2i = sp.tile([128, 32], F32, tag="b2i")
            nc.sync.dma_start(b2i[:], bias2i_h[:, :])
            b2h = sp.tile([128, 32], F32, tag="b2h")
            nc.sync.dma_start(b2h[:], bias2h_h[:, :])
            b2s = sp.tile([128, 32], F32, tag="b2s")
            nc.vector.tensor_tensor(b2s[:], b2i[:], b2h[:], ALU.add)
            c02s = sp.tile([128, 8], F32, tag="c02s")
            nc.sync.dma_start(c02s[:], c02c_h[:, :])
            onehs = sp.tile([1, 8], F32, tag="onehs")
            nc.sync.dma_start(onehs[:], oneh_h[:, :])
            abr = sp.tile([1, 512], F32, tag="abr")
            nc.sync.dma_start(abr[:], attbr_h[:, :])
            aw2 = sp.tile([128, 4], F32, tag="aw2")
            nc.sync.dma_start(aw2[:], attw2col[:, :])
            ab2 = sp.tile([1, 1], F32, tag="ab2")
            nc.sync.dma_start(ab2[:], attb2[:, :])
            scs = sp.tile([1, 2], F32, tag="scs")
            nc.sync.dma_start(scs[:], sconst[:, :])
            io4 = sp.tile([128, 4], F32, tag="io4")
            nc.sync.dma_start(io4[:], iota4[:, :])
            io4s = sp.tile([128, 4], F32, tag="io4s")
            nc.sync.dma_start(io4s[:], iotas[:, :])
            f1b = sp.tile([128, 1], F32, tag="f1b")
            nc.sync.dma_start(f1b[:], fc1b_col[:, :])
            onr = sp.tile([1, 128], F32, tag="onr")
            nc.sync.dma_start(onr[:], onesr[:, :])
            onc = sp.tile([128, 1], F32, tag="onc")
            nc.sync.dma_start(onc[:], onesc[:, :])
            # dram bounce buffers for collectives
            ar2_in = dp.tile([6144, 1], F32, tag="ar2_in")
            ar2_out = dp.tile([6144, 1], F32, tag="ar2_out")
            ar_in = dp.tile([1032, 1], F32, tag="ar_in")
            ar_out = dp.tile([1032, 1], F32, tag="ar_out")
            arl_in = dp.tile([128, VP // 128], F32, tag="arl_in")
            arl_out = dp.tile([128, VP // 128], F32, tag="arl_out")

            with tc.tile_pool(name="psA", bufs=3, space="PSUM") as psA:
                # ================= LSTM =================================
                # Layer 1: output-sharded (this core owns hidden units
                # 128c..128c+127 -> 512 gate rows).
                psg = psA.tile([1, 512], F32, tag="ps", name="psg1")
                for a in range(KZ // 2):
                    wc = wcchunks0[a]
                    for j in range(2):
                        t = 2 * a + j
                        z = xc[:, t:t + 1] if t < 8 else h0c[:, t - 8:t - 7]
                        nc.tensor.matmul(
                            psg[:, :], z, wc[:, 512 * j:512 * (j + 1)],
                            start=(t == 0), stop=(t == KZ - 1))
                gb = sp.tile([1, 512], F32, tag="gb")
                nc.vector.tensor_tensor(gb[:], psg[:, :], bsum[0:1, :], ALU.add)
                sg = sp.tile([1, 512], F32, tag="sg")
                nc.scalar.activation(sg[:], gb[:], AF.Sigmoid)
                tg = sp.tile([1, 128], F32, tag="tg")
                nc.scalar.activation(tg[:], gb[:, 256:384], AF.Tanh)
                t1 = sp.tile([1, 128], F32, tag="t1")
                nc.vector.tensor_tensor(t1[:], sg[:, 128:256], c0s[0:1, :], ALU.mult)
                t2 = sp.tile([1, 128], F32, tag="t2")
                nc.vector.tensor_tensor(t2[:], sg[:, 0:128], tg[:], ALU.mult)
                cn = sp.tile([1, 128], F32, tag="cn")
                nc.vector.tensor_tensor(cn[:], t1[:], t2[:], ALU.add)
                tcn = sp.tile([1, 128], F32, tag="tcn")
                nc.scalar.activation(tcn[:], cn[:], AF.Tanh)
                hn = sp.tile([1, 128], F32, tag="hn")
                nc.vector.tensor_tensor(hn[:], sg[:, 384:512], tcn[:], ALU.mult)
                # h1 shard as a column (lhsT for the layer-2 partial matvec)
                hncp = psA.tile([128, 1], F32, tag="ps", name="hncp")
                nc.tensor.matmul(hncp[:, :], hn[:], onr[0:1, 0:1], start=True, stop=True)
                hnc = sp.tile([128, 1], F32, tag="hnc")
                nc.vector.tensor_copy(hnc[:], hncp[:, :])

                # Layer 2 partial gates: contraction-sharded over h1 (and
                # h0), produced directly in (128, 32) column layout so the
                # AllReduce payload needs no post-transpose.
                g2cp = psA.tile([128, 32], F32, tag="ps", name="g2cp")
                for m in range(32):
                    r, mm = divmod(m, 16)
                    nc.tensor.matmul(g2cp[:, m:m + 1],
                                     w2sb[r][:, 128 * mm:128 * (mm + 1)],
                                     hnc[:, 0:1],
                                     start=True, stop=(not include_hh))
                    if include_hh:
                        nc.tensor.matmul(g2cp[:, m:m + 1],
                                         w2hb[r][:, 128 * mm:128 * (mm + 1)],
                                         h0shs[:, 0:1],
                                         start=False, stop=True)
                g2s = sp.tile([128, 32], F32, tag="g2s")
                nc.vector.tensor_copy(g2s[:], g2cp[:, :])

                # place [h1_shard | c1_shard] at block row c via one-hot matmul
                hncn = sp.tile([1, 256], F32, tag="hncn")
                nc.vector.tensor_copy(hncn[:, 0:128], hn[:])
                nc.vector.tensor_copy(hncn[:, 128:256], cn[:])
                phk = psA.tile([8, 256], F32, tag="ps", name="phk")
                nc.tensor.matmul(phk[:, :], onehs[:], hncn[:], start=True, stop=True)
                phs = sp.tile([8, 256], F32, tag="phs")
                nc.vector.tensor_copy(phs[:], phk[:, :])
                nc.scalar.dma_start(ar2_in[0:4096, :], g2s[:])
                nc.scalar.dma_start(ar2_in[4096:6144, :], phs[:])
                nc.gpsimd.collective_compute(
                    "AllReduce", ALU.add, replica_groups=RG,
                    ins=[ar2_in.opt()], outs=[ar2_out.opt()])

                # ---- post-AR: full gates2 + gathered h1/c1 ----
                agmark = [None]
                gcl = sp.tile([128, 32], F32, tag="gcl")
                ld_inst = nc.scalar.dma_start(
                    gcl[:],
                    ar2_out[0:4096, :].rearrange("(p t) q -> p (t q)", p=128))
                agmark[0] = ld_inst
                nc.scalar.dma_start(hc1blk[:, :], ar2_out[4096:6144, :])
                gc2 = sp.tile([128, 32], F32, tag="gc2")
                nc.vector.tensor_tensor(gc2[:], gcl[:], b2s[:], ALU.add)
                s2t = sp.tile([128, 32], F32, tag="s2t")
                nc.scalar.activation(s2t[:], gc2[:], AF.Sigmoid)
                tg2 = sp.tile([128, 8], F32, tag="tg2")
                nc.scalar.activation(tg2[:], gc2[:, 16:24], AF.Tanh)
                ft2 = sp.tile([128, 8], F32, tag="ft2")
                nc.vector.tensor_tensor(ft2[:], s2t[:, 8:16], c02s[:], ALU.mult)
                it2 = sp.tile([128, 8], F32, tag="it2")
                nc.vector.tensor_tensor(it2[:], s2t[:, 0:8], tg2[:], ALU.mult)
                cn2 = sp.tile([128, 8], F32, tag="cn2")
                nc.vector.tensor_tensor(cn2[:], ft2[:], it2[:], ALU.add)
                nc.scalar.dma_start(c2cols[:, :], cn2[:])
                tcn2 = sp.tile([128, 8], F32, tag="tcn2")
                nc.scalar.activation(tcn2[:], cn2[:], AF.Tanh)
                hn2 = sp.tile([128, 8], F32, tag="hn2")
                nc.vector.tensor_tensor(hn2[:], s2t[:, 24:32], tcn2[:], ALU.mult)
                nc.scalar.dma_start(h2cols[:, :], hn2[:])
                ht = hn2

                # ================= attention =============================
                # a1 = tanh(attw.T-style matvec), column layout (pattern B).
                # NOTE: PSUM accumulation groups are PE-global state and
                # cannot interleave -> each output column's k-accumulation
                # must complete before the next column starts (m outer).
                attc = []
                for a in range(2):
                    wc = attp.tile([128, 2048], F32, tag="attw", name=f"attw{a}")
                    nc.sync.dma_start(
                        wc[:].rearrange("p (j n) -> p j n", j=4),
                        attw[512 * a:512 * (a + 1), :].rearrange("(j p) n -> p j n", p=128))
                    attc.append(wc)
                # a1 as a row (pattern A), then transpose for the p matvec
                pa1 = psA.tile([1, 512], F32, tag="ps", name="pa1")
                for k in range(8):
                    a, j = divmod(k, 4)
                    nc.tensor.matmul(pa1[:, :], ht[:, k:k + 1],
                                     attc[a][:, 512 * j:512 * (j + 1)],
                                     start=(k == 0), stop=(k == 7))
                a1b = sp.tile([1, 512], F32, tag="a1b")
                nc.vector.tensor_tensor(a1b[:], pa1[:, :], abr[:], ALU.add)
                a1r = sp.tile([1, 512], F32, tag="a1r")
                nc.scalar.activation(a1r[:], a1b[:], AF.Tanh)
                a1cp = psA.tile([128, 4], F32, tag="ps", name="a1cp")
                for m in range(4):
                    nc.tensor.matmul(a1cp[:, m:m + 1],
                                     a1r[0:1, 128 * m:128 * (m + 1)],
                                     onr[0:1, 0:1], start=True, stop=True)
                a1s = sp.tile([128, 4], F32, tag="a1s")
                nc.vector.tensor_copy(a1s[:], a1cp[:, :])
                # p = S * sigmoid(a1 @ w2 + b2)
                pp = psA.tile([1, 1], F32, tag="ps")
                for m in range(4):
                    nc.tensor.matmul(pp[:, :], a1s[:, m:m + 1], aw2[:, m:m + 1],
                                     start=(m == 0), stop=(m == 3))
                psig = sp.tile([1, 1], F32, tag="psig")
                nc.scalar.activation(psig[:], pp[:, :], AF.Sigmoid, bias=ab2[0:1, 0:1])
                pv = sp.tile([1, 1], F32, tag="pv")
                nc.vector.tensor_scalar(pv[:], psig[:], scs[0:1, 0:1], None, ALU.mult)

                # wr = [we, ws, hi, p]  (rounded window bounds)
                wr = sp.tile([1, 4], F32, tag="wr")
                tA = sp.tile([1, 2], F32, tag="tA")
                tB = sp.tile([1, 2], F32, tag="tB")
                # raw ws -> tA[0:1], raw we -> tA[1:2]
                nc.vector.tensor_scalar(tA[:, 0:1], pv[:], 64.0, 0.0, ALU.subtract, ALU.max)
                nc.vector.tensor_scalar(tA[:, 1:2], pv[:], 64.0, scs[0:1, 1:2], ALU.add, ALU.min)
                # round to nearest (even): f32 -> int32 -> f32 convert
                ti = sp.tile([1, 2], mybir.dt.int32, tag="ti")
                nc.vector.tensor_copy(ti[:], tA[:, :])
                nc.vector.tensor_copy(tB[:, :], ti[:])
                # wr[1] = ws, wr[0] = we
                nc.vector.tensor_copy(wr[:, 1:2], tB[:, 0:1])
                nc.vector.tensor_copy(wr[:, 0:1], tB[:, 1:2])
                # hi = min(we, ws + 128)
                hi_t = sp.tile([1, 1], F32, tag="hi_t")
                nc.vector.tensor_scalar(hi_t[:], wr[:, 1:2], 128.0, None, ALU.add)
                nc.vector.tensor_tensor(wr[:, 2:3], hi_t[:], wr[:, 0:1], ALU.min)
                # wr[3] = -p/2048 (gauss exponent bias)
                nc.vector.tensor_scalar(wr[:, 3:4], pv[:], float(-1.0 / STD2), None, ALU.mult)
                nc.scalar.dma_start(aux[0:1, :], wr[:])
                # broadcast [ws, hi, -p/2048] to all partitions via K=1 matmul
                wbp = psA.tile([128, 3], F32, tag="ps")
                nc.tensor.matmul(wbp[:, :], onr[:], wr[:, 1:4], start=True, stop=True)
                wb = sp.tile([128, 3], F32, tag="wb")
                nc.vector.tensor_copy(wb[:], wbp[:, :])

                # scores (pattern B over encT shard), m outer for group safety
                encc = []
                for a in range(2):
                    ec = encTp.tile([128, 2048], F32, tag="encT", name=f"encT{a}")
                    nc.sync.dma_start(
                        ec[:].rearrange("p (j n) -> p j n", j=4),
                        encT[512 * a:512 * (a + 1), :].rearrange("(j p) n -> p j n", p=128))
                    encc.append(ec)
                psr = psA.tile([1, 512], F32, tag="ps", name="psr")
                for k in range(8):
                    a, j = divmod(k, 4)
                    nc.tensor.matmul(psr[:, :], ht[:, k:k + 1],
                                     encc[a][:, 512 * j:512 * (j + 1)],
                                     start=(k == 0), stop=(k == 7))
                scr = sp.tile([1, 512], F32, tag="scr")
                nc.vector.tensor_copy(scr[:], psr[:, :])
                sp4 = psA.tile([128, 4], F32, tag="ps", name="sp4")
                for m in range(4):
                    nc.tensor.matmul(sp4[:, m:m + 1],
                                     scr[0:1, 128 * m:128 * (m + 1)],
                                     onr[0:1, 0:1], start=True, stop=True)
                # masked exp, gauss weights
                tge = sp.tile([128, 4], F32, tag="tge")
                nc.vector.tensor_scalar(tge[:], io4[:], wb[:, 0:1], None, ALU.is_ge)
                tle = sp.tile([128, 4], F32, tag="tle")
                nc.vector.tensor_scalar(tle[:], io4[:], wb[:, 1:2], None, ALU.is_le)
                tmask = sp.tile([128, 4], F32, tag="tmask")
                nc.vector.tensor_tensor(tmask[:], tge[:], tle[:], ALU.mult)
                e4 = sp.tile([128, 4], F32, tag="e4")
                nc.scalar.activation(e4[:], sp4[:, :], AF.Exp)
                em = sp.tile([128, 4], F32, tag="em")
                nc.vector.tensor_tensor(em[:], e4[:], tmask[:], ALU.mult)
                zr = sp.tile([128, 1], F32, tag="zr")
                nc.vector.tensor_reduce(zr[:], em[:], mybir.AxisListType.X, ALU.add)
                # gauss = exp(iota/2048 - p/2048), bias from broadcast col
                gex = sp.tile([128, 4], F32, tag="gex")
                nc.scalar.activation(gex[:], io4s[:], AF.Exp, bias=wb[:, 2:3])
                w4 = sp.tile([128, 4], F32, tag="w4")
                nc.vector.tensor_tensor(w4[:], em[:], gex[:], ALU.mult)
                zp = psA.tile([1, 1], F32, tag="ps")
                nc.tensor.matmul(zp[:, :], zr[:], onc[:], start=True, stop=True)
                zq = sp.tile([1, 1], F32, tag="zq")
                nc.vector.tensor_copy(zq[:], zp[:, :])
                # ctx partial (pattern A over encN shard), n outer
                encn = []
                for a in range(2):
                    en = encNp.tile([128, 2048], F32, tag="encN", name=f"encN{a}")
                    di = nc.sync.dma_start(
                        en[:].rearrange("p (j n) -> p j n", j=2),
                        encN[256 * a:256 * (a + 1), :].rearrange("(j p) n -> p j n", p=128))
                    bass._add_dep_helper(di.ins, agmark[0].ins, sync=True,
                                         reason="stage encN after AG1")
                    encn.append(en)
                ctxp = psA.tile([1, 1024], F32, tag="ps")
                for n in range(2):
                    for k in range(4):
                        a, j = divmod(k, 2)
                        nc.tensor.matmul(
                            ctxp[:, 512 * n:512 * (n + 1)],
                            w4[:, k:k + 1],
                            encn[a][:, 1024 * j + 512 * n:1024 * j + 512 * (n + 1)],
                            start=(k == 0), stop=(k == 3))
                ctxs = sp.tile([1, 1024], F32, tag="ctxs")
                nc.vector.tensor_copy(ctxs[:], ctxp[:, :])
                nc.scalar.dma_start(ar_in[0:1024, :], ctxs[:])
                nc.scalar.dma_start(ar_in[1024:1025, :], zq[:])
                nc.scalar.dma_start(ar_in[1025:1032, :], zpd[:, 0:7])
                nc.gpsimd.collective_compute(
                    "AllReduce", ALU.add, replica_groups=RG,
                    ins=[ar_in.opt()], outs=[ar_out.opt()])
                # 1/Z and normalized outputs
                zt = sp.tile([1, 1], F32, tag="zt")
                nc.scalar.dma_start(zt[:], ar_out[1024:1025, :])
                rz = sp.tile([1, 1], F32, tag="rz")
                nc.vector.reciprocal(rz[:], zt[:])
                rzp = psA.tile([128, 1], F32, tag="ps")
                nc.tensor.matmul(rzp[:, :], onr[:], rz[:], start=True, stop=True)
                rb = sp.tile([128, 1], F32, tag="rb")
                nc.vector.tensor_copy(rb[:], rzp[:, :])
                a4 = sp.tile([128, 4], F32, tag="a4")
                nc.vector.tensor_scalar(a4[:], w4[:], rb[:, 0:1], None, ALU.mult)
                nc.scalar.dma_start(a_part[:, :], a4[:])
                ctxrow = sp.tile([1, 1024], F32, tag="ctxrow")
                nc.scalar.dma_start(
                    ctxrow[:],
                    ar_out[0:1024, :].rearrange("(o p) q -> o (p q)", o=1))
                ccp = psA.tile([128, 8], F32, tag="ps", name="ccp")
                for t in range(8):
                    nc.tensor.matmul(ccp[:, t:t + 1],
                                     ctxrow[0:1, 128 * t:128 * (t + 1)],
                                     onr[0:1, 0:1], start=True, stop=True)
                ctxn = sp.tile([128, 8], F32, tag="ctxn")
                nc.vector.tensor_scalar(ctxn[:], ccp[:, :], rb[:, 0:1], None, ALU.mult)

                # ================= fc1 ===================================
                op_ = psA.tile([128, 1], F32, tag="ps")
                for a in range(2):
                    fct = fc1p.tile([128, 1024], F32, tag="fc1w")
                    di = nc.sync.dma_start(
                        fct[:].rearrange("p (j n) -> p j n", j=8),
                        fc1w[1024 * a:1024 * (a + 1), :].rearrange("(j p) n -> p j n", p=128))
                    bass._add_dep_helper(di.ins, agmark[0].ins, sync=True,
                                         reason="stage fc1w after AG1")
                    for j in range(8):
                        k = 8 * a + j
                        z = ctxn[:, k:k + 1] if k < 8 else ht[:, k - 8:k - 7]
                        nc.tensor.matmul(op_[:, :], fct[:, 128 * j:128 * (j + 1)], z,
                                         start=(k == 0), stop=(k == 15))
                oc = sp.tile([128, 1], F32, tag="oc")
                nc.scalar.activation(oc[:], op_[:, :], AF.Tanh, bias=f1b[:, 0:1])
                nc.scalar.dma_start(out_part[:, :], oc[:])
                ocb = sp.tile([128, 1], BF16, tag="ocb")
                nc.vector.tensor_copy(ocb[:], oc[:])

            # ================= fc2 + log_softmax =========================
            # fc2 is sharded over the CONTRACTION (each core owns 128 rows
            # of `out` and the matching 128 rows of fc2_w.T for the FULL
            # padded vocab). Partial logits live as (128, 256) columns,
            # summed across cores with one AllReduce; log-softmax then
            # runs fully local on every core.
            NT = VP // 128   # 256 column tiles
            with tc.tile_pool(name="psB", bufs=1, space="PSUM") as psB:
                fkc = []
                for ch in range(8):
                    fk = fc2p.tile([128, NT // 8 * 128], BF16, tag="fc2w_w",
                                   name=f"fc2w{ch}", bufs=6 if include_hh else 8)
                    di = nc.sync.dma_start(fk[:], fc2w[:, 4096 * ch:4096 * (ch + 1)])
                    bass._add_dep_helper(di.ins, agmark[0].ins, sync=True,
                                         reason="stage fc2w after AR_A")
                    fkc.append(fk)
                Lp = psB.tile([128, NT], F32, tag="L")
                for n in range(NT):
                    ch, o = divmod(n, 32)
                    nc.tensor.matmul(Lp[:, n:n + 1],
                                     fkc[ch][:, 128 * o:128 * (o + 1)],
                                     ocb[:, 0:1], start=True, stop=True)
                Ls = sp.tile([128, NT], F32, tag="Ls")
                nc.vector.tensor_copy(Ls[:], Lp[:, :])
                nc.scalar.dma_start(arl_in[:, :], Ls[:])
                nc.gpsimd.collective_compute(
                    "AllReduce", ALU.add, replica_groups=RG,
                    ins=[arl_in.opt()], outs=[arl_out.opt()])
                La = sp.tile([128, NT], F32, tag="La")
                nc.scalar.dma_start(La[:], arl_out[:, :])
                bcols = sp.tile([128, NT], F32, tag="bcols")
                nc.sync.dma_start(bcols[:], fc2b[:, :])
                Lb = sp.tile([128, NT], F32, tag="Lb")
                nc.vector.tensor_tensor(Lb[:], La[:], bcols[:], ALU.add)
                ex = sp.tile([128, NT], F32, tag="ex")
                zcol = sp.tile([128, 1], F32, tag="zcol")
                nc.scalar.activation(ex[:], Lb[:], AF.Exp, accum_out=zcol[:])
                zps = psB.tile([1, 1], F32, tag="zps", name="zps")
                nc.tensor.matmul(zps[:, :], zcol[:], onc[:], start=True, stop=True)
                lgz = sp.tile([1, 1], F32, tag="lgz")
                nc.scalar.activation(lgz[:], zps[:, :], AF.Ln)
                lgb = psB.tile([128, 1], F32, tag="lgb", name="lgb")
                nc.tensor.matmul(lgb[:, :], onr[:], lgz[:], start=True, stop=True)
                lgs = sp.tile([128, 1], F32, tag="lgs")
                nc.vector.tensor_copy(lgs[:], lgb[:, :])
                ys = sp.tile([128, NT], F32, tag="ys")
                nc.vector.tensor_scalar(ys[:], Lb[:], lgs[:, 0:1], None, ALU.subtract)
                nc.scalar.dma_start(y_part[:, :], ys[:, :])

    _split_sync_waits(nc)
    return nc


_NC_CACHE = {}


def _get_nc(include_hh: bool):
    if include_hh not in _NC_CACHE:
        _NC_CACHE[include_hh] = _build(include_hh)
    return _NC_CACHE[include_hh]


# --------------------------------------------------------------------------
def _host_prep(inputs, include_hh):
    emb = np.asarray(inputs["embedding"], np.float32)
    word = int(np.asarray(inputs["word"]).reshape(-1)[0])
    x = emb[word]
    Sf = float(np.asarray(inputs["source_sentence_length"]))
    h0 = np.asarray(inputs["h0"], np.float32)
    c0 = np.asarray(inputs["c0"], np.float32)
    w_ih = np.asarray(inputs["lstm_w_ih"], np.float32)
    w_hh = np.asarray(inputs["lstm_w_hh"], np.float32)
    b_ih = np.asarray(inputs["lstm_b_ih"], np.float32)
    b_hh = np.asarray(inputs["lstm_b_hh"], np.float32)
    enc = np.ascontiguousarray(np.asarray(inputs["encoder_output"], np.float32)[:, 0, :])
    att1w = np.asarray(inputs["att_fc1_w"], np.float32)
    att1b = np.asarray(inputs["att_fc1_b"], np.float32)
    att2w = np.asarray(inputs["att_fc2_w"], np.float32)
    att2b = np.asarray(inputs["att_fc2_b"], np.float32)
    fc1w = np.asarray(inputs["fc1_w"], np.float32)
    fc1b = np.asarray(inputs["fc1_b"], np.float32)
    fc2w = np.asarray(inputs["fc2_w"], np.float32)
    fc2b = np.asarray(inputs["fc2_b"], np.float32)

    fc2w_p = np.zeros((VP, H), np.float32)
    fc2w_p[:V] = fc2w
    fc2b_p = np.full((VP,), NEG, np.float32)
    fc2b_p[:V] = fc2b
    encTf = np.ascontiguousarray(enc.T)
    attwT = np.ascontiguousarray(att1w.T)
    fc1wT = np.ascontiguousarray(fc1w.T)

    in_maps = []
    for c in range(NCORE):
        u = slice(128 * c, 128 * (c + 1))
        gr = np.concatenate([np.arange(128 * c, 128 * (c + 1)) + 1024 * g for g in range(4)])
        d = {}
        d["xcols"] = np.ascontiguousarray(x.reshape(8, 128).T)
        if include_hh:
            d["h0cols"] = np.ascontiguousarray(
                np.concatenate([h0[l, 0].reshape(8, 128).T for l in range(L)], axis=1))
        d["c0row"] = np.ascontiguousarray(c0[0, 0, u].reshape(1, 128))
        d["c02c"] = np.ascontiguousarray(c0[1, 0].reshape(8, 128).T)  # (128,8)
        if include_hh:
            wc0 = np.concatenate([w_ih[0][gr].T, w_hh[0][gr].T])
        else:
            wc0 = w_ih[0][gr].T
        d["wcat0"] = np.ascontiguousarray(wc0)
        # layer 2: contraction-sharded -> own 128 h-rows of W2^T, all gates
        if include_hh:
            wc1 = np.concatenate([w_ih[1].T[u, :], w_hh[1].T[u, :]])  # (256, 4096)
        else:
            wc1 = w_ih[1].T[u, :]                                      # (128, 4096)
        d["wcat1"] = np.ascontiguousarray(wc1)
        d["biasg"] = np.ascontiguousarray(
            np.concatenate([b_ih[0][gr], b_hh[0][gr]]).reshape(1, 1024))
        d["bias2i"] = np.ascontiguousarray(b_ih[1].reshape(32, 128).T)
        d["bias2h"] = np.ascontiguousarray(b_hh[1].reshape(32, 128).T)
        d["oneh"] = np.zeros((1, 8), np.float32)
        d["oneh"][0, c] = 1.0
        if include_hh:
            d["h0sh"] = np.ascontiguousarray(h0[1, 0, u].reshape(128, 1))
        d["attw"] = attwT
        d["attb_col"] = np.ascontiguousarray(att1b.reshape(4, 128).T)
        d["attbr"] = np.ascontiguousarray(att1b.reshape(1, 512))
        d["attw2col"] = np.ascontiguousarray(att2w.reshape(-1).reshape(4, 128).T)
        d["attb2"] = np.ascontiguousarray(att2b.reshape(1, 1))
        d["sconst"] = np.array([[Sf, Sf - 1.0]], np.float32)
        d["iota4"] = np.ascontiguousarray(
            (512 * c + np.arange(512, dtype=np.float32)).reshape(4, 128).T)
        d["iotas"] = np.ascontiguousarray(d["iota4"] / np.float32(2048.0))
        d["encT"] = np.ascontiguousarray(encTf[:, 512 * c:512 * (c + 1)])
        d["encN"] = np.ascontiguousarray(enc[512 * c:512 * (c + 1)])
        d["fc1w"] = np.ascontiguousarray(fc1wT[:, u])
        d["fc1b_col"] = np.ascontiguousarray(fc1b[u].reshape(128, 1))
        d["fc2w"] = np.ascontiguousarray(fc2w_p[:, u].T).astype(bfloat16)
        d["fc2b"] = np.ascontiguousarray(fc2b_p.reshape(VP // 128, 128).T)
        d["onesr"] = np.ones((1, 128), np.float32)
        d["onesc"] = np.ones((128, 1), np.float32)
        d["zpad"] = np.zeros((1, 8), np.float32)
        in_maps.append(d)
    return in_maps


def _unshard(results, inputs):
    y = results[0]["y_part"].T.reshape(-1)[:V]
    out_vec = np.concatenate([results[c]["out_part"][:, 0] for c in range(NCORE)])
    blk = results[0]["hc1blk"].reshape(8, 256)
    h1 = blk[:, 0:128].reshape(-1)
    c1 = blk[:, 128:256].reshape(-1)
    h2 = results[0]["h2cols"].T.reshape(-1)
    c2 = results[0]["c2cols"].T.reshape(-1)
    h_n = np.stack([h1, h2])
    c_n = np.stack([c1, c2])
    a_full = np.concatenate([results[c]["a_part"].T.reshape(-1) for c in range(NCORE)])
    we = int(results[0]["aux"][0, 0])
    ws = int(results[0]["aux"][0, 1])
    Sv = int(np.asarray(inputs["source_sentence_length"]))
    idx = ws + np.arange(WIN)
    valid = idx <= we
    a = np.where(valid, a_full[np.clip(idx, 0, Sv - 1)], 0.0).astype(np.float32)
    return (
        y.reshape(1, 1, V).astype(np.float32),
        out_vec.reshape(1, 1, H).astype(np.float32),
        h_n[:, None, :].astype(np.float32),
        c_n[:, None, :].astype(np.float32),
        a.reshape(1, 1, WIN),
    )


def kernel(**inputs):
    h0 = np.asarray(inputs["h0"])
    include_hh = bool(np.any(h0 != 0))
    nc = _get_nc(include_hh)
    in_maps = _host_prep(inputs, include_hh)
    res = run_bass_kernel_spmd(nc, in_maps, core_ids=list(range(NCORE)))
    return _unshard(res.results, inputs)


# revision 34
# speedup vs baseline: 1.0106x; 1.0106x over previous
"""Trainium2 8-core Bass kernel for nn_Decoder (single-step LSTM decoder with
Gaussian-windowed attention and a 32k-vocab log-softmax head).

Sharding strategy (tensor-parallel over all heavy weights):
  - LSTM: hidden units sharded 128/core; gate rows [i,f,g,o] for this core's
    units gathered into a (2048, 512) transposed weight block per layer.
    AllGather (128 floats/core) of h after each layer.
  - Attention: p-network replicated (tiny); encoder positions sharded
    512/core, window mask computed densely; AllReduce of
    [ctx_partial(1024), Z_partial(1)].
  - fc1: output units sharded 128/core, AllGather of `out`.
  - fc2/log_softmax: vocab padded to 32768 and sharded 4096/core; the
    softmax denominator is AllGathered (8 scalars) and summed locally.

All matvecs run on the TensorEngine with host-pre-transposed weights so the
contraction dim lands on SBUF partitions with fully contiguous DMA lines.
"""

import numpy as np
from ml_dtypes import bfloat16
import concourse.bass as bass
import concourse.mybir as mybir
from concourse import tile
from concourse.bass_utils import run_bass_kernel_spmd

NCORE = 8
H = 1024
L = 2
V = 32000
VP = 32768          # vocab padded to 8*4096
VS = VP // NCORE    # 4096 per core
S = 4096
WW = 64
WIN = 2 * WW + 1
STD2 = 2.0 * (WW / 2.0) ** 2   # 2048
NEG = -1e30
F32 = mybir.dt.float32
BF16 = mybir.dt.bfloat16
AF = mybir.ActivationFunctionType
ALU = mybir.AluOpType
RG = [list(range(NCORE))]


# --------------------------------------------------------------------------
# Workaround: the walrus build in this container rejects instructions with
# more than ONE sync-wait command. Waits are AND-conditions evaluated on an
# in-order engine queue, so excess waits are moved onto NoOps inserted
# immediately before the instruction.
def _split_sync_waits(nc, max_waits=1):
    for fn in nc.m.functions:
        for blk in fn.blocks:
            instrs = list(blk.instructions)
            new_instrs = []
            changed = False
            for ins in instrs:
                si = ins.sync_info
                waits = list(si.on_wait) if si is not None else []
                if len(waits) > max_waits:
                    extra = waits[:-max_waits]
                    keep = waits[-max_waits:]
                    for j, w in enumerate(extra):
                        nop = mybir.InstNoOp(
                            name=f"{ins.name}-wsplit{j}", ins=[], outs=[],
                            sync_info=mybir.SyncInfo(on_wait=[w], on_update=[]),
                        )
                        nop.engine = ins.engine
                        new_instrs.append(nop)
                    ins.sync_info = mybir.SyncInfo(
                        on_wait=keep, on_update=list(si.on_update))
                    changed = True
                new_instrs.append(ins)
            if changed:
                blk.instructions = new_instrs


# --------------------------------------------------------------------------
def _build(include_hh: bool):
    nc = bass.Bass(num_devices=NCORE)
    KZ = 16 if include_hh else 8        # contraction k-tiles per LSTM layer
    WCROWS = 128 * KZ

    def inp(name, shape):
        return nc.dram_tensor(name, shape, F32, kind="ExternalInput")

    xcols = inp("xcols", [128, 8])
    h0cols = inp("h0cols", [128, 16]) if include_hh else None
    c0row = inp("c0row", [1, 128])
    wcat0_h = inp("wcat0", [WCROWS, 512])
    wcat1_h = inp("wcat1", [128 * (2 if include_hh else 1), 4096])
    bias2i_h = inp("bias2i", [128, 32])
    bias2h_h = inp("bias2h", [128, 32])
    c02c_h = inp("c02c", [128, 8])
    oneh_h = inp("oneh", [1, 8])
    h0sh_h = inp("h0sh", [128, 1]) if include_hh else None
    biasg = inp("biasg", [1, 1024])
    attw = inp("attw", [1024, 512])
    attb_col = inp("attb_col", [128, 4])
    attbr_h = inp("attbr", [1, 512])
    attw2col = inp("attw2col", [128, 4])
    attb2 = inp("attb2", [1, 1])
    sconst = inp("sconst", [1, 2])
    iota4 = inp("iota4", [128, 4])
    iotas = inp("iotas", [128, 4])
    encT = inp("encT", [1024, 512])
    encN = inp("encN", [512, 1024])
    fc1w = inp("fc1w", [2048, 128])
    fc1b_col = inp("fc1b_col", [128, 1])
    fc2w = nc.dram_tensor("fc2w", [128, VP], BF16, kind="ExternalInput")
    fc2b = inp("fc2b", [128, VP // 128])
    onesr = inp("onesr", [1, 128])
    onesc = inp("onesc", [128, 1])
    zpad = inp("zpad", [1, 8])

    y_part = nc.dram_tensor("y_part", [128, VP // 128], F32, kind="ExternalOutput")
    out_part = nc.dram_tensor("out_part", [128, 1], F32, kind="ExternalOutput")
    hc1blk = nc.dram_tensor("hc1blk", [2048, 1], F32, kind="ExternalOutput")
    h2cols = nc.dram_tensor("h2cols", [128, 8], F32, kind="ExternalOutput")
    c2cols = nc.dram_tensor("c2cols", [128, 8], F32, kind="ExternalOutput")
    a_part = nc.dram_tensor("a_part", [128, 4], F32, kind="ExternalOutput")
    aux = nc.dram_tensor("aux", [1, 4], F32, kind="ExternalOutput")

    with tile.TileContext(nc) as tc:
        with (
            tc.tile_pool(name="smalls", bufs=1) as sp,
            tc.tile_pool(name="wcatp", bufs=3) as wcatp,
            tc.tile_pool(name="attp", bufs=2) as attp,
            tc.tile_pool(name="encTp", bufs=2) as encTp,
            tc.tile_pool(name="encNp", bufs=2) as encNp,
            tc.tile_pool(name="fc1p", bufs=2) as fc1p,
            tc.tile_pool(name="fc2p", bufs=8) as fc2p,
            tc.tile_pool(name="dram", bufs=1, space="DRAM") as dp,
        ):
            # ---------- collective warm-up (absorbs first-CC staging) ----
            zpd = sp.tile([1, 8], F32, tag="zpd")
            nc.sync.dma_start(zpd[:], zpad[:, :])
            warm_in = dp.tile([6144, 1], F32, tag="warm_in")
            warm_out = dp.tile([6144, 1], F32, tag="warm_out")
            nc.scalar.dma_start(warm_in[0:8, :], zpd[:, :])
            nc.gpsimd.collective_compute(
                "AllReduce", ALU.add, replica_groups=RG,
                ins=[warm_in.opt()], outs=[warm_out.opt()])

            # ---------- tiny LSTM activations first (matmul lhsT inputs) --
            xc = sp.tile([128, 8], F32, tag="xc")
            nc.sync.dma_start(xc[:], xcols[:, :])
            if include_hh:
                h0c = sp.tile([128, 16], F32, tag="h0c")
                nc.sync.dma_start(h0c[:], h0cols[:, :])

            # ---------- LSTM weights stream first (critical path head) ----
            # layer 0 in small (128,1024) chunks so the first matmul can
            # start as early as possible; layer 1 in (128,2048) chunks.
            wcchunks0 = []
            for a in range(KZ // 2):
                wc = wcatp.tile([128, 1024], F32, tag="wcat0",
                                name=f"wcat0_{a}", bufs=4 if include_hh else KZ // 2)
                nc.sync.dma_start(
                    wc[:].rearrange("p (j n) -> p j n", j=2),
                    wcat0_h[256 * a:256 * (a + 1), :].rearrange(
                        "(j p) n -> p j n", p=128))
                wcchunks0.append(wc)
            # layer-2 weights: own 128 contraction rows x all 4096 gates
            w2sb = []
            for r in range(2):
                wt = wcatp.tile([128, 2048], F32, tag="wcat",
                                name=f"w2sb{r}", bufs=2)
                nc.sync.dma_start(wt[:], wcat1_h[0:128, 2048 * r:2048 * (r + 1)])
                w2sb.append(wt)
            if include_hh:
                w2hb = []
                for r in range(2):
                    wt = wcatp.tile([128, 2048], F32, tag="wcath",
                                    name=f"w2hb{r}", bufs=2)
                    nc.sync.dma_start(wt[:], wcat1_h[128:256, 2048 * r:2048 * (r + 1)])
                    w2hb.append(wt)
                h0shs = sp.tile([128, 1], F32, tag="h0shs")
                nc.sync.dma_start(h0shs[:], h0sh_h[:, :])

            # ---------- small resident inputs ----
            c0s = sp.tile([1, 128], F32, tag="c0s")
            nc.sync.dma_start(c0s[:], c0row[:, :])
            bgs = sp.tile([1, 1024], F32, tag="bgs")
            nc.sync.dma_start(bgs[:], biasg[:, :])
            bsum = sp.tile([1, 512], F32, tag="bsum")
            nc.vector.tensor_tensor(bsum[:, :], bgs[0:1, 0:512], bgs[0:1, 512:1024], ALU.add)
            b2i = sp.tile([128, 32], F32, tag="b2i")
            nc.sync.dma_start(b2i[:], bias2i_h[:, :])
            b2h = sp.tile([128, 32], F32, tag="b2h")
            nc.sync.dma_start(b2h[:], bias2h_h[:, :])
            b2s = sp.tile([128, 32], F32, tag="b2s")
            nc.vector.tensor_tensor(b2s[:], b2i[:], b2h[:], ALU.add)
            c02s = sp.tile([128, 8], F32, tag="c02s")
            nc.sync.dma_start(c02s[:], c02c_h[:, :])
            onehs = sp.tile([1, 8], F32, tag="onehs")
            nc.sync.dma_start(onehs[:], oneh_h[:, :])
            abr = sp.tile([1, 512], F32, tag="abr")
            nc.sync.dma_start(abr[:], attbr_h[:, :])
            aw2 = sp.tile([128, 4], F32, tag="aw2")
            nc.sync.dma_start(aw2[:], attw2col[:, :])
            ab2 = sp.tile([1, 1], F32, tag="ab2")
            nc.sync.dma_start(ab2[:], attb2[:, :])
            scs = sp.tile([1, 2], F32, tag="scs")
            nc.sync.dma_start(scs[:], sconst[:, :])
            io4 = sp.tile([128, 4], F32, tag="io4")
            nc.sync.dma_start(io4[:], iota4[:, :])
            io4s = sp.tile([128, 4], F32, tag="io4s")
            nc.sync.dma_start(io4s[:], iotas[:, :])
            f1b = sp.tile([128, 1], F32, tag="f1b")
            nc.sync.dma_start(f1b[:], fc1b_col[:, :])
            onr = sp.tile([1, 128], F32, tag="onr")
            nc.sync.dma_start(onr[:], onesr[:, :])
            onc = sp.tile([128, 1], F32, tag="onc")
            nc.sync.dma_start(onc[:], onesc[:, :])
            # dram bounce buffers for collectives
            ar2_in = dp.tile([6144, 1], F32, tag="ar2_in")
            ar2_out = dp.tile([6144, 1], F32, tag="ar2_out")
            ar_in = dp.tile([1032, 1], F32, tag="ar_in")
            ar_out = dp.tile([1032, 1], F32, tag="ar_out")
            arl_in = dp.tile([128, VP // 128], F32, tag="arl_in")
            arl_out = dp.tile([128, VP // 128], F32, tag="arl_out")

            with tc.tile_pool(name="psA", bufs=3, space="PSUM") as psA:
                # ================= LSTM =================================
                # Layer 1: output-sharded (this core owns hidden units
                # 128c..128c+127 -> 512 gate rows).
                psg = psA.tile([1, 512], F32, tag="ps", name="psg1")
                for a in range(KZ // 2):
                    wc = wcchunks0[a]
                    for j in range(2):
                        t = 2 * a + j
                        z = xc[:, t:t + 1] if t < 8 else h0c[:, t - 8:t - 7]
                        nc.tensor.matmul(
                            psg[:, :], z, wc[:, 512 * j:512 * (j + 1)],
                            start=(t == 0), stop=(t == KZ - 1))
                gb = sp.tile([1, 512], F32, tag="gb")
                nc.vector.tensor_tensor(gb[:], psg[:, :], bsum[0:1, :], ALU.add)
                sg = sp.tile([1, 512], F32, tag="sg")
                nc.scalar.activation(sg[:], gb[:], AF.Sigmoid)
                tg = sp.tile([1, 128], F32, tag="tg")
                nc.scalar.activation(tg[:], gb[:, 256:384], AF.Tanh)
                t1 = sp.tile([1, 128], F32, tag="t1")
                nc.vector.tensor_tensor(t1[:], sg[:, 128:256], c0s[0:1, :], ALU.mult)
                t2 = sp.tile([1, 128], F32, tag="t2")
                nc.vector.tensor_tensor(t2[:], sg[:, 0:128], tg[:], ALU.mult)
                cn = sp.tile([1, 128], F32, tag="cn")
                nc.vector.tensor_tensor(cn[:], t1[:], t2[:], ALU.add)
                tcn = sp.tile([1, 128], F32, tag="tcn")
                nc.scalar.activation(tcn[:], cn[:], AF.Tanh)
                hn = sp.tile([1, 128], F32, tag="hn")
                nc.vector.tensor_tensor(hn[:], sg[:, 384:512], tcn[:], ALU.mult)
                # h1 shard as a column (lhsT for the layer-2 partial matvec)
                hncp = psA.tile([128, 1], F32, tag="ps", name="hncp")
                nc.tensor.matmul(hncp[:, :], hn[:], onr[0:1, 0:1], start=True, stop=True)
                hnc = sp.tile([128, 1], F32, tag="hnc")
                nc.vector.tensor_copy(hnc[:], hncp[:, :])

                # Layer 2 partial gates: contraction-sharded over h1 (and
                # h0), produced directly in (128, 32) column layout so the
                # AllReduce payload needs no post-transpose.
                g2cp = psA.tile([128, 32], F32, tag="ps", name="g2cp")
                for m in range(32):
                    r, mm = divmod(m, 16)
                    nc.tensor.matmul(g2cp[:, m:m + 1],
                                     w2sb[r][:, 128 * mm:128 * (mm + 1)],
                                     hnc[:, 0:1],
                                     start=True, stop=(not include_hh))
                    if include_hh:
                        nc.tensor.matmul(g2cp[:, m:m + 1],
                                         w2hb[r][:, 128 * mm:128 * (mm + 1)],
                                         h0shs[:, 0:1],
                                         start=False, stop=True)
                g2s = sp.tile([128, 32], F32, tag="g2s")
                nc.vector.tensor_copy(g2s[:], g2cp[:, :])

                # place [h1_shard | c1_shard] at block row c via one-hot matmul
                hncn = sp.tile([1, 256], F32, tag="hncn")
                nc.vector.tensor_copy(hncn[:, 0:128], hn[:])
                nc.vector.tensor_copy(hncn[:, 128:256], cn[:])
                phk = psA.tile([8, 256], F32, tag="ps", name="phk")
                nc.tensor.matmul(phk[:, :], onehs[:], hncn[:], start=True, stop=True)
                phs = sp.tile([8, 256], F32, tag="phs")
                nc.vector.tensor_copy(phs[:], phk[:, :])
                nc.scalar.dma_start(ar2_in[0:4096, :], g2s[:])
                nc.scalar.dma_start(ar2_in[4096:6144, :], phs[:])
                nc.gpsimd.collective_compute(
                    "AllReduce", ALU.add, replica_groups=RG,
                    ins=[ar2_in.opt()], outs=[ar2_out.opt()])

                # ---- post-AR: full gates2 + gathered h1/c1 ----
                agmark = [None]
                gcl = sp.tile([128, 32], F32, tag="gcl")
                ld_inst = nc.scalar.dma_start(
                    gcl[:],
                    ar2_out[0:4096, :].rearrange("(p t) q -> p (t q)", p=128))
                agmark[0] = ld_inst
                nc.scalar.dma_start(hc1blk[:, :], ar2_out[4096:6144, :])
                gc2 = sp.tile([128, 32], F32, tag="gc2")
                nc.vector.tensor_tensor(gc2[:], gcl[:], b2s[:], ALU.add)
                s2t = sp.tile([128, 32], F32, tag="s2t")
                nc.scalar.activation(s2t[:], gc2[:], AF.Sigmoid)
                tg2 = sp.tile([128, 8], F32, tag="tg2")
                nc.scalar.activation(tg2[:], gc2[:, 16:24], AF.Tanh)
                ft2 = sp.tile([128, 8], F32, tag="ft2")
                nc.vector.tensor_tensor(ft2[:], s2t[:, 8:16], c02s[:], ALU.mult)
                it2 = sp.tile([128, 8], F32, tag="it2")
                nc.vector.tensor_tensor(it2[:], s2t[:, 0:8], tg2[:], ALU.mult)
                cn2 = sp.tile([128, 8], F32, tag="cn2")
                nc.vector.tensor_tensor(cn2[:], ft2[:], it2[:], ALU.add)
                nc.scalar.dma_start(c2cols[:, :], cn2[:])
                tcn2 = sp.tile([128, 8], F32, tag="tcn2")
                nc.scalar.activation(tcn2[:], cn2[:], AF.Tanh)
                hn2 = sp.tile([128, 8], F32, tag="hn2")
                nc.vector.tensor_tensor(hn2[:], s2t[:, 24:32], tcn2[:], ALU.mult)
                nc.scalar.dma_start(h2cols[:, :], hn2[:])
                ht = hn2

                # ================= attention =============================
                # a1 = tanh(attw.T-style matvec), column layout (pattern B).
                # NOTE: PSUM accumulation groups are PE-global state and
                # cannot interleave -> each output column's k-accumulation
                # must complete before the next column starts (m outer).
                attc = []
                for a in range(2):
                    wc = attp.tile([128, 2048], F32, tag="attw", name=f"attw{a}")
                    nc.sync.dma_start(
                        wc[:].rearrange("p (j n) -> p j n", j=4),
                        attw[512 * a:512 * (a + 1), :].rearrange("(j p) n -> p j n", p=128))
                    attc.append(wc)
                # a1 as a row (pattern A), then transpose for the p matvec
                pa1 = psA.tile([1, 512], F32, tag="ps", name="pa1")
                for k in range(8):
                    a, j = divmod(k, 4)
                    nc.tensor.matmul(pa1[:, :], ht[:, k:k + 1],
                                     attc[a][:, 512 * j:512 * (j + 1)],
                                     start=(k == 0), stop=(k == 7))
                a1b = sp.tile([1, 512], F32, tag="a1b")
                nc.vector.tensor_tensor(a1b[:], pa1[:, :], abr[:], ALU.add)
                a1r = sp.tile([1, 512], F32, tag="a1r")
                nc.scalar.activation(a1r[:], a1b[:], AF.Tanh)
                a1cp = psA.tile([128, 4], F32, tag="ps", name="a1cp")
                for m in range(4):
                    nc.tensor.matmul(a1cp[:, m:m + 1],
                                     a1r[0:1, 128 * m:128 * (m + 1)],
                                     onr[0:1, 0:1], start=True, stop=True)
                a1s = sp.tile([128, 4], F32, tag="a1s")
                nc.vector.tensor_copy(a1s[:], a1cp[:, :])
                # p = S * sigmoid(a1 @ w2 + b2)
                pp = psA.tile([1, 1], F32, tag="ps")
                for m in range(4):
                    nc.tensor.matmul(pp[:, :], a1s[:, m:m + 1], aw2[:, m:m + 1],
                                     start=(m == 0), stop=(m == 3))
                psig = sp.tile([1, 1], F32, tag="psig")
                nc.scalar.activation(psig[:], pp[:, :], AF.Sigmoid, bias=ab2[0:1, 0:1])
                pv = sp.tile([1, 1], F32, tag="pv")
                nc.vector.tensor_scalar(pv[:], psig[:], scs[0:1, 0:1], None, ALU.mult)

                # wr = [we, ws, hi, p]  (rounded window bounds)
                wr = sp.tile([1, 4], F32, tag="wr")
                tA = sp.tile([1, 2], F32, tag="tA")
                tB = sp.tile([1, 2], F32, tag="tB")
                # raw ws -> tA[0:1], raw we -> tA[1:2]
                nc.vector.tensor_scalar(tA[:, 0:1], pv[:], 64.0, 0.0, ALU.subtract, ALU.max)
                nc.vector.tensor_scalar(tA[:, 1:2], pv[:], 64.0, scs[0:1, 1:2], ALU.add, ALU.min)
                # round to nearest (even): f32 -> int32 -> f32 convert
                ti = sp.tile([1, 2], mybir.dt.int32, tag="ti")
                nc.vector.tensor_copy(ti[:], tA[:, :])
                nc.vector.tensor_copy(tB[:, :], ti[:])
                # wr[1] = ws, wr[0] = we
                nc.vector.tensor_copy(wr[:, 1:2], tB[:, 0:1])
                nc.vector.tensor_copy(wr[:, 0:1], tB[:, 1:2])
                # hi = min(we, ws + 128)
                hi_t = sp.tile([1, 1], F32, tag="hi_t")
                nc.vector.tensor_scalar(hi_t[:], wr[:, 1:2], 128.0, None, ALU.add)
                nc.vector.tensor_tensor(wr[:, 2:3], hi_t[:], wr[:, 0:1], ALU.min)
                # wr[3] = -p/2048 (gauss exponent bias)
                nc.vector.tensor_scalar(wr[:, 3:4], pv[:], float(-1.0 / STD2), None, ALU.mult)
                nc.scalar.dma_start(aux[0:1, :], wr[:])
                # broadcast [ws, hi, -p/2048] to all partitions via K=1 matmul
                wbp = psA.tile([128, 3], F32, tag="ps")
                nc.tensor.matmul(wbp[:, :], onr[:], wr[:, 1:4], start=True, stop=True)
                wb = sp.tile([128, 3], F32, tag="wb")
                nc.vector.tensor_copy(wb[:], wbp[:, :])

                # scores (pattern B over encT shard), m outer for group safety
                encc = []
                for a in range(2):
                    ec = encTp.tile([128, 2048], F32, tag="encT", name=f"encT{a}")
                    nc.sync.dma_start(
                        ec[:].rearrange("p (j n) -> p j n", j=4),
                        encT[512 * a:512 * (a + 1), :].rearrange("(j p) n -> p j n", p=128))
                    encc.append(ec)
                psr = psA.tile([1, 512], F32, tag="ps", name="psr")
                for k in range(8):
                    a, j = divmod(k, 4)
                    nc.tensor.matmul(psr[:, :], ht[:, k:k + 1],
                                     encc[a][:, 512 * j:512 * (j + 1)],
                                     start=(k == 0), stop=(k == 7))
                scr = sp.tile([1, 512], F32, tag="scr")
                nc.vector.tensor_copy(scr[:], psr[:, :])
                sp4 = psA.tile([128, 4], F32, tag="ps", name="sp4")
                for m in range(4):
                    nc.tensor.matmul(sp4[:, m:m + 1],
                                     scr[0:1, 128 * m:128 * (m + 1)],
                                     onr[0:1, 0:1], start=True, stop=True)
                # masked exp, gauss weights
                tge = sp.tile([128, 4], F32, tag="tge")
                nc.vector.tensor_scalar(tge[:], io4[:], wb[:, 0:1], None, ALU.is_ge)
                tle = sp.tile([128, 4], F32, tag="tle")
                nc.vector.tensor_scalar(tle[:], io4[:], wb[:, 1:2], None, ALU.is_le)
                tmask = sp.tile([128, 4], F32, tag="tmask")
                nc.vector.tensor_tensor(tmask[:], tge[:], tle[:], ALU.mult)
                e4 = sp.tile([128, 4], F32, tag="e4")
                nc.scalar.activation(e4[:], sp4[:, :], AF.Exp)
                em = sp.tile([128, 4], F32, tag="em")
                nc.vector.tensor_tensor(em[:], e4[:], tmask[:], ALU.mult)
                zr = sp.tile([128, 1], F32, tag="zr")
                nc.vector.tensor_reduce(zr[:], em[:], mybir.AxisListType.X, ALU.add)
                # gauss = exp(iota/2048 - p/2048), bias from broadcast col
                gex = sp.tile([128, 4], F32, tag="gex")
                nc.scalar.activation(gex[:], io4s[:], AF.Exp, bias=wb[:, 2:3])
                w4 = sp.tile([128, 4], F32, tag="w4")
                nc.vector.tensor_tensor(w4[:], em[:], gex[:], ALU.mult)
                zp = psA.tile([1, 1], F32, tag="ps")
                nc.tensor.matmul(zp[:, :], zr[:], onc[:], start=True, stop=True)
                zq = sp.tile([1, 1], F32, tag="zq")
                nc.vector.tensor_copy(zq[:], zp[:, :])
                # ctx partial (pattern A over encN shard), n outer
                encn = []
                for a in range(2):
                    en = encNp.tile([128, 2048], F32, tag="encN", name=f"encN{a}")
                    di = nc.sync.dma_start(
                        en[:].rearrange("p (j n) -> p j n", j=2),
                        encN[256 * a:256 * (a + 1), :].rearrange("(j p) n -> p j n", p=128))
                    bass._add_dep_helper(di.ins, agmark[0].ins, sync=True,
                                         reason="stage encN after AG1")
                    encn.append(en)
                ctxp = psA.tile([1, 1024], F32, tag="ps")
                for n in range(2):
                    for k in range(4):
                        a, j = divmod(k, 2)
                        nc.tensor.matmul(
                            ctxp[:, 512 * n:512 * (n + 1)],
                            w4[:, k:k + 1],
                            encn[a][:, 1024 * j + 512 * n:1024 * j + 512 * (n + 1)],
                            start=(k == 0), stop=(k == 3))
                ctxs = sp.tile([1, 1024], F32, tag="ctxs")
                nc.vector.tensor_copy(ctxs[:], ctxp[:, :])
                nc.scalar.dma_start(ar_in[0:1024, :], ctxs[:])
                nc.scalar.dma_start(ar_in[1024:1025, :], zq[:])
                nc.scalar.dma_start(ar_in[1025:1032, :], zpd[:, 0:7])
                nc.gpsimd.collective_compute(
                    "AllReduce", ALU.add, replica_groups=RG,
                    ins=[ar_in.opt()], outs=[ar_out.opt()])
                # 1/Z and normalized outputs
                zt = sp.tile([1, 1], F32, tag="zt")
                nc.scalar.dma_start(zt[:], ar_out[1024:1025, :])
                rz = sp.tile([1, 1], F32, tag="rz")
                nc.vector.reciprocal(rz[:], zt[:])
                rzp = psA.tile([128, 1], F32, tag="ps")
                nc.tensor.matmul(rzp[:, :], onr[:], rz[:], start=True, stop=True)
                rb = sp.tile([128, 1], F32, tag="rb")
                nc.vector.tensor_copy(rb[:], rzp[:, :])
                a4 = sp.tile([128, 4], F32, tag="a4")
                nc.vector.tensor_scalar(a4[:], w4[:], rb[:, 0:1], None, ALU.mult)
                nc.scalar.dma_start(a_part[:, :], a4[:])
                ctxrow = sp.tile([1, 1024], F32, tag="ctxrow")
                nc.scalar.dma_start(
                    ctxrow[:],
                    ar_out[0:1024, :].rearrange("(o p) q -> o (p q)", o=1))
                ccp = psA.tile([128, 8], F32, tag="ps", name="ccp")
                for t in range(8):
                    nc.tensor.matmul(ccp[:, t:t + 1],
                                     ctxrow[0:1, 128 * t:128 * (t + 1)],
                                     onr[0:1, 0:1], start=True, stop=True)
                ctxn = sp.tile([128, 8], F32, tag="ctxn")
                nc.vector.tensor_scalar(ctxn[:], ccp[:, :], rb[:, 0:1], None, ALU.mult)

                # ================= fc1 ===================================
                op_ = psA.tile([128, 1], F32, tag="ps")
                for a in range(2):
                    fct = fc1p.tile([128, 1024], F32, tag="fc1w")
                    di = nc.sync.dma_start(
                        fct[:].rearrange("p (j n) -> p j n", j=8),
                        fc1w[1024 * a:1024 * (a + 1), :].rearrange("(j p) n -> p j n", p=128))
                    bass._add_dep_helper(di.ins, agmark[0].ins, sync=True,
                                         reason="stage fc1w after AG1")
                    for j in range(8):
                        k = 8 * a + j
                        z = ctxn[:, k:k + 1] if k < 8 else ht[:, k - 8:k - 7]
                        nc.tensor.matmul(op_[:, :], fct[:, 128 * j:128 * (j + 1)], z,
                                         start=(k == 0), stop=(k == 15))
                oc = sp.tile([128, 1], F32, tag="oc")
                nc.scalar.activation(oc[:], op_[:, :], AF.Tanh, bias=f1b[:, 0:1])
                nc.scalar.dma_start(out_part[:, :], oc[:])
                ocb = sp.tile([128, 1], BF16, tag="ocb")
                nc.vector.tensor_copy(ocb[:], oc[:])

            # ================= fc2 + log_softmax =========================
            # fc2 is sharded over the CONTRACTION (each core owns 128 rows
            # of `out` and the matching 128 rows of fc2_w.T for the FULL
            # padded vocab). Partial logits live as (128, 256) columns,
            # summed across cores with one AllReduce; log-softmax then
            # runs fully local on every core.
            NT = VP // 128   # 256 column tiles
            with tc.tile_pool(name="psB", bufs=1, space="PSUM") as psB:
                fkc = []
                for ch in range(8):
                    fk = fc2p.tile([128, NT // 8 * 128], BF16, tag="fc2w_w",
                                   name=f"fc2w{ch}", bufs=6 if include_hh else 8)
                    di = nc.sync.dma_start(fk[:], fc2w[:, 4096 * ch:4096 * (ch + 1)])
                    bass._add_dep_helper(di.ins, agmark[0].ins, sync=True,
                                         reason="stage fc2w after AR_A")
                    fkc.append(fk)
                Lp = psB.tile([128, NT], F32, tag="L")
                for n in range(NT):
                    ch, o = divmod(n, 32)
                    nc.tensor.matmul(Lp[:, n:n + 1],
                                     fkc[ch][:, 128 * o:128 * (o + 1)],
                                     ocb[:, 0:1], start=True, stop=True)
                Ls = sp.tile([128, NT], F32, tag="Ls")
                nc.vector.tensor_copy(Ls[:], Lp[:, :])
                nc.scalar.dma_start(arl_in[:, :], Ls[:])
                nc.gpsimd.collective_compute(
                    "AllReduce", ALU.add, replica_groups=RG,
                    ins=[arl_in.opt()], outs=[arl_out.opt()])
                La = sp.tile([128, NT], F32, tag="La")
                nc.scalar.dma_start(La[:], arl_out[:, :])
                bcols = sp.tile([128, NT], F32, tag="bcols")
                nc.sync.dma_start(bcols[:], fc2b[:, :])
                Lb = sp.tile([128, NT], F32, tag="Lb")
                nc.vector.tensor_tensor(Lb[:], La[:], bcols[:], ALU.add)
                ex = sp.tile([128, NT], F32, tag="ex")
                zcol = sp.tile([128, 1], F32, tag="zcol")
                nc.scalar.activation(ex[:], Lb[:], AF.Exp, accum_out=zcol[:])
                zps = psB.tile([1, 1], F32, tag="zps", name="zps")
                nc.tensor.matmul(zps[:, :], zcol[:], onc[:], start=True, stop=True)
                lgz = sp.tile([1, 1], F32, tag="lgz")
                nc.scalar.activation(lgz[:], zps[:, :], AF.Ln)
                lgb = psB.tile([128, 1], F32, tag="lgb", name="lgb")
                nc.tensor.matmul(lgb[:, :], onr[:], lgz[:], start=True, stop=True)
                lgs = sp.tile([128, 1], F32, tag="lgs")
                nc.vector.tensor_copy(lgs[:], lgb[:, :])
                ys = sp.tile([128, NT], F32, tag="ys")
                nc.vector.tensor_scalar(ys[:], Lb[:], lgs[:, 0:1], None, ALU.subtract)
                nc.scalar.dma_start(y_part[:, :], ys[:, :])

    _split_sync_waits(nc)
    return nc


_NC_CACHE = {}


def _get_nc(include_hh: bool):
    if include_hh not in _NC_CACHE:
        _NC_CACHE[include_hh] = _build(include_hh)
    return _NC_CACHE[include_hh]


# --------------------------------------------------------------------------
def _host_prep(inputs, include_hh):
    emb = np.asarray(inputs["embedding"], np.float32)
    word = int(np.asarray(inputs["word"]).reshape(-1)[0])
    x = emb[word]
    Sf = float(np.asarray(inputs["source_sentence_length"]))
    h0 = np.asarray(inputs["h0"], np.float32)
    c0 = np.asarray(inputs["c0"], np.float32)
    w_ih = np.asarray(inputs["lstm_w_ih"], np.float32)
    w_hh = np.asarray(inputs["lstm_w_hh"], np.float32)
    b_ih = np.asarray(inputs["lstm_b_ih"], np.float32)
    b_hh = np.asarray(inputs["lstm_b_hh"], np.float32)
    enc = np.ascontiguousarray(np.asarray(inputs["encoder_output"], np.float32)[:, 0, :])
    att1w = np.asarray(inputs["att_fc1_w"], np.float32)
    att1b = np.asarray(inputs["att_fc1_b"], np.float32)
    att2w = np.asarray(inputs["att_fc2_w"], np.float32)
    att2b = np.asarray(inputs["att_fc2_b"], np.float32)
    fc1w = np.asarray(inputs["fc1_w"], np.float32)
    fc1b = np.asarray(inputs["fc1_b"], np.float32)
    fc2w = np.asarray(inputs["fc2_w"], np.float32)
    fc2b = np.asarray(inputs["fc2_b"], np.float32)

    fc2w_p = np.zeros((VP, H), np.float32)
    fc2w_p[:V] = fc2w
    fc2b_p = np.full((VP,), NEG, np.float32)
    fc2b_p[:V] = fc2b
    encTf = np.ascontiguousarray(enc.T)
    attwT = np.ascontiguousarray(att1w.T)
    fc1wT = np.ascontiguousarray(fc1w.T)

    in_maps = []
    for c in range(NCORE):
        u = slice(128 * c, 128 * (c + 1))
        gr = np.concatenate([np.arange(128 * c, 128 * (c + 1)) + 1024 * g for g in range(4)])
        d = {}
        d["xcols"] = np.ascontiguousarray(x.reshape(8, 128).T)
        if include_hh:
            d["h0cols"] = np.ascontiguousarray(
                np.concatenate([h0[l, 0].reshape(8, 128).T for l in range(L)], axis=1))
        d["c0row"] = np.ascontiguousarray(c0[0, 0, u].reshape(1, 128))
        d["c02c"] = np.ascontiguousarray(c0[1, 0].reshape(8, 128).T)  # (128,8)
        if include_hh:
            wc0 = np.concatenate([w_ih[0][gr].T, w_hh[0][gr].T])
        else:
            wc0 = w_ih[0][gr].T
        d["wcat0"] = np.ascontiguousarray(wc0)
        # layer 2: contraction-sharded -> own 128 h-rows of W2^T, all gates
        if include_hh:
            wc1 = np.concatenate([w_ih[1].T[u, :], w_hh[1].T[u, :]])  # (256, 4096)
        else:
            wc1 = w_ih[1].T[u, :]                                      # (128, 4096)
        d["wcat1"] = np.ascontiguousarray(wc1)
        d["biasg"] = np.ascontiguousarray(
            np.concatenate([b_ih[0][gr], b_hh[0][gr]]).reshape(1, 1024))
        d["bias2i"] = np.ascontiguousarray(b_ih[1].reshape(32, 128).T)
        d["bias2h"] = np.ascontiguousarray(b_hh[1].reshape(32, 128).T)
        d["oneh"] = np.zeros((1, 8), np.float32)
        d["oneh"][0, c] = 1.0
        if include_hh:
            d["h0sh"] = np.ascontiguousarray(h0[1, 0, u].reshape(128, 1))
        d["attw"] = attwT
        d["attb_col"] = np.ascontiguousarray(att1b.reshape(4, 128).T)
        d["attbr"] = np.ascontiguousarray(att1b.reshape(1, 512))
        d["attw2col"] = np.ascontiguousarray(att2w.reshape(-1).reshape(4, 128).T)
        d["attb2"] = np.ascontiguousarray(att2b.reshape(1, 1))
        d["sconst"] = np.array([[Sf, Sf - 1.0]], np.float32)
        d["iota4"] = np.ascontiguousarray(
            (512 * c + np.arange(512, dtype=np.float32)).reshape(4, 128).T)
        d["iotas"] = np.ascontiguousarray(d["iota4"] / np.float32(2048.0))
        d["encT"] = np.ascontiguousarray(encTf[:, 512 * c:512 * (c + 1)])
        d["encN"] = np.ascontiguousarray(enc[512 * c:512 * (c + 1)])
        d["fc1w"] = np.ascontiguousarray(fc1wT[:, u])
        d["fc1b_col"] = np.ascontiguousarray(fc1b[u].reshape(128, 1))
        d["fc2w"] = np.ascontiguousarray(fc2w_p[:, u].T).astype(bfloat16)
        d["fc2b"] = np.ascontiguousarray(fc2b_p.reshape(VP // 128, 128).T)
        d["onesr"] = np.ones((1, 128), np.float32)
        d["onesc"] = np.ones((128, 1), np.float32)
        d["zpad"] = np.zeros((1, 8), np.float32)
        in_maps.append(d)
    return in_maps


def _unshard(results, inputs):
    y = results[0]["y_part"].T.reshape(-1)[:V]
    out_vec = np.concatenate([results[c]["out_part"][:, 0] for c in range(NCORE)])
    blk = results[0]["hc1blk"].reshape(8, 256)
    h1 = blk[:, 0:128].reshape(-1)
    c1 = blk[:, 128:256].reshape(-1)
    h2 = results[0]["h2cols"].T.reshape(-1)
    c2 = results[0]["c2cols"].T.reshape(-1)
    h_n = np.stack([h1, h2])
    c_n = np.stack([c1, c2])
    a_full = np.concatenate([results[c]["a_part"].T.reshape(-1) for c in range(NCORE)])
    we = int(results[0]["aux"][0, 0])
    ws = int(results[0]["aux"][0, 1])
    Sv = int(np.asarray(inputs["source_sentence_length"]))
    idx = ws + np.arange(WIN)
    valid = idx <= we
    a = np.where(valid, a_full[np.clip(idx, 0, Sv - 1)], 0.0).astype(np.float32)
    return (
        y.reshape(1, 1, V).astype(np.float32),
        out_vec.reshape(1, 1, H).astype(np.float32),
        h_n[:, None, :].astype(np.float32),
        c_n[:, None, :].astype(np.float32),
        a.reshape(1, 1, WIN),
    )


def kernel(**inputs):
    h0 = np.asarray(inputs["h0"])
    include_hh = bool(np.any(h0 != 0))
    nc = _get_nc(include_hh)
    in_maps = _host_prep(inputs, include_hh)
    res = run_bass_kernel_spmd(nc, in_maps, core_ids=list(range(NCORE)))
    return _unshard(res.results, inputs)


# revision 35
# speedup vs baseline: 1.1298x; 1.1180x over previous
"""Trainium2 8-core Bass kernel for nn_Decoder (single-step LSTM decoder with
Gaussian-windowed attention and a 32k-vocab log-softmax head).

Sharding strategy (tensor-parallel over all heavy weights):
  - LSTM: hidden units sharded 128/core; gate rows [i,f,g,o] for this core's
    units gathered into a (2048, 512) transposed weight block per layer.
    AllGather (128 floats/core) of h after each layer.
  - Attention: p-network replicated (tiny); encoder positions sharded
    512/core, window mask computed densely; AllReduce of
    [ctx_partial(1024), Z_partial(1)].
  - fc1: output units sharded 128/core, AllGather of `out`.
  - fc2/log_softmax: vocab padded to 32768 and sharded 4096/core; the
    softmax denominator is AllGathered (8 scalars) and summed locally.

All matvecs run on the TensorEngine with host-pre-transposed weights so the
contraction dim lands on SBUF partitions with fully contiguous DMA lines.
"""

import numpy as np
from ml_dtypes import bfloat16
import concourse.bass as bass
import concourse.mybir as mybir
from concourse import tile
from concourse.bass_utils import run_bass_kernel_spmd

NCORE = 8
H = 1024
L = 2
V = 32000
VP = 32768          # vocab padded to 8*4096
VS = VP // NCORE    # 4096 per core
S = 4096
WW = 64
WIN = 2 * WW + 1
STD2 = 2.0 * (WW / 2.0) ** 2   # 2048
NEG = -1e30
F32 = mybir.dt.float32
BF16 = mybir.dt.bfloat16
AF = mybir.ActivationFunctionType
ALU = mybir.AluOpType
RG = [list(range(NCORE))]


# --------------------------------------------------------------------------
# Workaround: the walrus build in this container rejects instructions with
# more than ONE sync-wait command. Waits are AND-conditions evaluated on an
# in-order engine queue, so excess waits are moved onto NoOps inserted
# immediately before the instruction.
def _split_sync_waits(nc, max_waits=1):
    for fn in nc.m.functions:
        for blk in fn.blocks:
            instrs = list(blk.instructions)
            new_instrs = []
            changed = False
            for ins in instrs:
                si = ins.sync_info
                waits = list(si.on_wait) if si is not None else []
                if len(waits) > max_waits:
                    extra = waits[:-max_waits]
                    keep = waits[-max_waits:]
                    for j, w in enumerate(extra):
                        nop = mybir.InstNoOp(
                            name=f"{ins.name}-wsplit{j}", ins=[], outs=[],
                            sync_info=mybir.SyncInfo(on_wait=[w], on_update=[]),
                        )
                        nop.engine = ins.engine
                        new_instrs.append(nop)
                    ins.sync_info = mybir.SyncInfo(
                        on_wait=keep, on_update=list(si.on_update))
                    changed = True
                new_instrs.append(ins)
            if changed:
                blk.instructions = new_instrs


# --------------------------------------------------------------------------
def _build(include_hh: bool):
    nc = bass.Bass(num_devices=NCORE)
    KZ = 16 if include_hh else 8        # contraction k-tiles per LSTM layer
    WCROWS = 128 * KZ

    def inp(name, shape):
        return nc.dram_tensor(name, shape, F32, kind="ExternalInput")

    xcols = inp("xcols", [128, 8])
    h0cols = inp("h0cols", [128, 16]) if include_hh else None
    c0row = inp("c0row", [1, 128])
    wcat0_h = inp("wcat0", [WCROWS, 512])
    wcat1_h = inp("wcat1", [128 * (2 if include_hh else 1), 4096])
    bias2i_h = inp("bias2i", [128, 32])
    bias2h_h = inp("bias2h", [128, 32])
    c02c_h = inp("c02c", [128, 8])
    oneh_h = inp("oneh", [1, 8])
    h0sh_h = inp("h0sh", [128, 1]) if include_hh else None
    biasg = inp("biasg", [1, 1024])
    attw = inp("attw", [1024, 512])
    attb_col = inp("attb_col", [128, 4])
    attbr_h = inp("attbr", [1, 512])
    attw2col = inp("attw2col", [128, 4])
    attb2 = inp("attb2", [1, 1])
    sconst = inp("sconst", [1, 2])
    iota4 = inp("iota4", [128, 4])
    iotas = inp("iotas", [128, 4])
    encT = inp("encT", [1024, 512])
    encN = inp("encN", [512, 1024])
    fc1w = inp("fc1w", [2048, 128])
    fc1b_col = inp("fc1b_col", [128, 1])
    fc2w = nc.dram_tensor("fc2w", [128, VP], BF16, kind="ExternalInput")
    fc2b = inp("fc2b", [128, VP // 128])
    onesr = inp("onesr", [1, 128])
    onesc = inp("onesc", [128, 1])
    zpad = inp("zpad", [1, 8])

    y_part = nc.dram_tensor("y_part", [128, VP // 128], F32, kind="ExternalOutput")
    out_part = nc.dram_tensor("out_part", [128, 1], F32, kind="ExternalOutput")
    hc1blk = nc.dram_tensor("hc1blk", [2048, 1], F32, kind="ExternalOutput")
    h2cols = nc.dram_tensor("h2cols", [128, 8], F32, kind="ExternalOutput")
    c2cols = nc.dram_tensor("c2cols", [128, 8], F32, kind="ExternalOutput")
    a_part = nc.dram_tensor("a_part", [128, 4], F32, kind="ExternalOutput")
    aux = nc.dram_tensor("aux", [1, 4], F32, kind="ExternalOutput")

    with tile.TileContext(nc) as tc:
        with (
            tc.tile_pool(name="smalls", bufs=1) as sp,
            tc.tile_pool(name="wcatp", bufs=3) as wcatp,
            tc.tile_pool(name="attp", bufs=2) as attp,
            tc.tile_pool(name="encTp", bufs=2) as encTp,
            tc.tile_pool(name="encNp", bufs=2) as encNp,
            tc.tile_pool(name="fc1p", bufs=2) as fc1p,
            tc.tile_pool(name="fc2p", bufs=8) as fc2p,
            tc.tile_pool(name="dram", bufs=1, space="DRAM") as dp,
        ):
            # ---------- collective warm-up (absorbs first-CC staging) ----
            zpd = sp.tile([1, 8], F32, tag="zpd")
            nc.sync.dma_start(zpd[:], zpad[:, :])
            warm_in = dp.tile([6144, 1], F32, tag="warm_in")
            warm_out = dp.tile([6144, 1], F32, tag="warm_out")
            nc.scalar.dma_start(warm_in[0:8, :], zpd[:, :])
            nc.gpsimd.collective_compute(
                "AllReduce", ALU.add, replica_groups=RG,
                ins=[warm_in.opt()], outs=[warm_out.opt()])

            # ---------- tiny LSTM activations first (matmul lhsT inputs) --
            xc = sp.tile([128, 8], F32, tag="xc")
            nc.sync.dma_start(xc[:], xcols[:, :])
            if include_hh:
                h0c = sp.tile([128, 16], F32, tag="h0c")
                nc.sync.dma_start(h0c[:], h0cols[:, :])

            # ---------- LSTM weights stream first (critical path head) ----
            # layer 0 in small (128,1024) chunks so the first matmul can
            # start as early as possible; layer 1 in (128,2048) chunks.
            wcchunks0 = []
            for a in range(KZ):
                wc = wcatp.tile([128, 512], F32, tag="wcat0",
                                name=f"wcat0_{a}", bufs=8 if include_hh else KZ)
                nc.sync.dma_start(wc[:], wcat0_h[128 * a:128 * (a + 1), :])
                wcchunks0.append(wc)
            # layer-2 weights: own 128 contraction rows x all 4096 gates
            w2sb = []
            for r in range(2):
                wt = wcatp.tile([128, 2048], F32, tag="wcat",
                                name=f"w2sb{r}", bufs=2)
                nc.sync.dma_start(wt[:], wcat1_h[0:128, 2048 * r:2048 * (r + 1)])
                w2sb.append(wt)
            if include_hh:
                w2hb = []
                for r in range(2):
                    wt = wcatp.tile([128, 2048], F32, tag="wcath",
                                    name=f"w2hb{r}", bufs=2)
                    nc.sync.dma_start(wt[:], wcat1_h[128:256, 2048 * r:2048 * (r + 1)])
                    w2hb.append(wt)
                h0shs = sp.tile([128, 1], F32, tag="h0shs")
                nc.sync.dma_start(h0shs[:], h0sh_h[:, :])

            # ---------- small resident inputs ----
            c0s = sp.tile([1, 128], F32, tag="c0s")
            nc.sync.dma_start(c0s[:], c0row[:, :])
            bgs = sp.tile([1, 1024], F32, tag="bgs")
            nc.sync.dma_start(bgs[:], biasg[:, :])
            bsum = sp.tile([1, 512], F32, tag="bsum")
            nc.vector.tensor_tensor(bsum[:, :], bgs[0:1, 0:512], bgs[0:1, 512:1024], ALU.add)
            b2i = sp.tile([128, 32], F32, tag="b2i")
            nc.sync.dma_start(b2i[:], bias2i_h[:, :])
            b2h = sp.tile([128, 32], F32, tag="b2h")
            nc.sync.dma_start(b2h[:], bias2h_h[:, :])
            b2s = sp.tile([128, 32], F32, tag="b2s")
            nc.vector.tensor_tensor(b2s[:], b2i[:], b2h[:], ALU.add)
            c02s = sp.tile([128, 8], F32, tag="c02s")
            nc.sync.dma_start(c02s[:], c02c_h[:, :])
            onehs = sp.tile([1, 8], F32, tag="onehs")
            nc.sync.dma_start(onehs[:], oneh_h[:, :])
            abr = sp.tile([1, 512], F32, tag="abr")
            nc.sync.dma_start(abr[:], attbr_h[:, :])
            aw2 = sp.tile([128, 4], F32, tag="aw2")
            nc.sync.dma_start(aw2[:], attw2col[:, :])
            ab2 = sp.tile([1, 1], F32, tag="ab2")
            nc.sync.dma_start(ab2[:], attb2[:, :])
            scs = sp.tile([1, 2], F32, tag="scs")
            nc.sync.dma_start(scs[:], sconst[:, :])
            io4 = sp.tile([128, 4], F32, tag="io4")
            nc.sync.dma_start(io4[:], iota4[:, :])
            io4s = sp.tile([128, 4], F32, tag="io4s")
            nc.sync.dma_start(io4s[:], iotas[:, :])
            f1b = sp.tile([128, 1], F32, tag="f1b")
            nc.sync.dma_start(f1b[:], fc1b_col[:, :])
            onr = sp.tile([1, 128], F32, tag="onr")
            nc.sync.dma_start(onr[:], onesr[:, :])
            onc = sp.tile([128, 1], F32, tag="onc")
            nc.sync.dma_start(onc[:], onesc[:, :])
            # dram bounce buffers for collectives
            ar2_in = dp.tile([6144, 1], F32, tag="ar2_in")
            ar2_out = dp.tile([6144, 1], F32, tag="ar2_out")
            ar_in = dp.tile([1032, 1], F32, tag="ar_in")
            ar_out = dp.tile([1032, 1], F32, tag="ar_out")
            arl_in = dp.tile([128, VP // 128], BF16, tag="arl_in")
            arl_out = dp.tile([128, VP // 128], BF16, tag="arl_out")

            with tc.tile_pool(name="psA", bufs=3, space="PSUM") as psA:
                # ================= LSTM =================================
                # Layer 1: output-sharded (this core owns hidden units
                # 128c..128c+127 -> 512 gate rows).
                psg = psA.tile([1, 512], F32, tag="ps", name="psg1")
                for t in range(KZ):
                    z = xc[:, t:t + 1] if t < 8 else h0c[:, t - 8:t - 7]
                    nc.tensor.matmul(
                        psg[:, :], z, wcchunks0[t][:, :],
                        start=(t == 0), stop=(t == KZ - 1))
                gb = sp.tile([1, 512], F32, tag="gb")
                nc.vector.tensor_tensor(gb[:], psg[:, :], bsum[0:1, :], ALU.add)
                sg = sp.tile([1, 512], F32, tag="sg")
                nc.scalar.activation(sg[:], gb[:], AF.Sigmoid)
                tg = sp.tile([1, 128], F32, tag="tg")
                nc.scalar.activation(tg[:], gb[:, 256:384], AF.Tanh)
                t1 = sp.tile([1, 128], F32, tag="t1")
                nc.vector.tensor_tensor(t1[:], sg[:, 128:256], c0s[0:1, :], ALU.mult)
                t2 = sp.tile([1, 128], F32, tag="t2")
                nc.vector.tensor_tensor(t2[:], sg[:, 0:128], tg[:], ALU.mult)
                cn = sp.tile([1, 128], F32, tag="cn")
                nc.vector.tensor_tensor(cn[:], t1[:], t2[:], ALU.add)
                tcn = sp.tile([1, 128], F32, tag="tcn")
                nc.scalar.activation(tcn[:], cn[:], AF.Tanh)
                hn = sp.tile([1, 128], F32, tag="hn")
                nc.vector.tensor_tensor(hn[:], sg[:, 384:512], tcn[:], ALU.mult)
                # h1 shard as a column (lhsT for the layer-2 partial matvec)
                hncp = psA.tile([128, 1], F32, tag="ps", name="hncp")
                nc.tensor.matmul(hncp[:, :], hn[:], onr[0:1, 0:1], start=True, stop=True)
                hnc = sp.tile([128, 1], F32, tag="hnc")
                nc.vector.tensor_copy(hnc[:], hncp[:, :])

                # Layer 2 partial gates: contraction-sharded over h1 (and
                # h0), produced directly in (128, 32) column layout so the
                # AllReduce payload needs no post-transpose.
                g2cp = psA.tile([128, 32], F32, tag="ps", name="g2cp")
                for m in range(32):
                    r, mm = divmod(m, 16)
                    nc.tensor.matmul(g2cp[:, m:m + 1],
                                     w2sb[r][:, 128 * mm:128 * (mm + 1)],
                                     hnc[:, 0:1],
                                     start=True, stop=(not include_hh))
                    if include_hh:
                        nc.tensor.matmul(g2cp[:, m:m + 1],
                                         w2hb[r][:, 128 * mm:128 * (mm + 1)],
                                         h0shs[:, 0:1],
                                         start=False, stop=True)
                g2s = sp.tile([128, 32], F32, tag="g2s")
                nc.vector.tensor_copy(g2s[:], g2cp[:, :])

                # place [h1_shard | c1_shard] at block row c via one-hot matmul
                hncn = sp.tile([1, 256], F32, tag="hncn")
                nc.vector.tensor_copy(hncn[:, 0:128], hn[:])
                nc.vector.tensor_copy(hncn[:, 128:256], cn[:])
                phk = psA.tile([8, 256], F32, tag="ps", name="phk")
                nc.tensor.matmul(phk[:, :], onehs[:], hncn[:], start=True, stop=True)
                phs = sp.tile([8, 256], F32, tag="phs")
                nc.vector.tensor_copy(phs[:], phk[:, :])
                nc.scalar.dma_start(ar2_in[0:4096, :], g2s[:])
                nc.scalar.dma_start(ar2_in[4096:6144, :], phs[:])
                nc.gpsimd.collective_compute(
                    "AllReduce", ALU.add, replica_groups=RG,
                    ins=[ar2_in.opt()], outs=[ar2_out.opt()])

                # ---- post-AR: full gates2 + gathered h1/c1 ----
                agmark = [None]
                gcl = sp.tile([128, 32], F32, tag="gcl")
                ld_inst = nc.scalar.dma_start(
                    gcl[:],
                    ar2_out[0:4096, :].rearrange("(p t) q -> p (t q)", p=128))
                agmark[0] = ld_inst
                nc.scalar.dma_start(hc1blk[:, :], ar2_out[4096:6144, :])
                gc2 = sp.tile([128, 32], F32, tag="gc2")
                nc.vector.tensor_tensor(gc2[:], gcl[:], b2s[:], ALU.add)
                s2t = sp.tile([128, 32], F32, tag="s2t")
                nc.scalar.activation(s2t[:], gc2[:], AF.Sigmoid)
                tg2 = sp.tile([128, 8], F32, tag="tg2")
                nc.scalar.activation(tg2[:], gc2[:, 16:24], AF.Tanh)
                ft2 = sp.tile([128, 8], F32, tag="ft2")
                nc.vector.tensor_tensor(ft2[:], s2t[:, 8:16], c02s[:], ALU.mult)
                it2 = sp.tile([128, 8], F32, tag="it2")
                nc.vector.tensor_tensor(it2[:], s2t[:, 0:8], tg2[:], ALU.mult)
                cn2 = sp.tile([128, 8], F32, tag="cn2")
                nc.vector.tensor_tensor(cn2[:], ft2[:], it2[:], ALU.add)
                nc.scalar.dma_start(c2cols[:, :], cn2[:])
                tcn2 = sp.tile([128, 8], F32, tag="tcn2")
                nc.scalar.activation(tcn2[:], cn2[:], AF.Tanh)
                hn2 = sp.tile([128, 8], F32, tag="hn2")
                nc.vector.tensor_tensor(hn2[:], s2t[:, 24:32], tcn2[:], ALU.mult)
                nc.scalar.dma_start(h2cols[:, :], hn2[:])
                ht = hn2

                # ================= attention =============================
                # a1 = tanh(attw.T-style matvec), column layout (pattern B).
                # NOTE: PSUM accumulation groups are PE-global state and
                # cannot interleave -> each output column's k-accumulation
                # must complete before the next column starts (m outer).
                attc = []
                for a in range(2):
                    wc = attp.tile([128, 2048], F32, tag="attw", name=f"attw{a}")
                    nc.sync.dma_start(
                        wc[:].rearrange("p (j n) -> p j n", j=4),
                        attw[512 * a:512 * (a + 1), :].rearrange("(j p) n -> p j n", p=128))
                    attc.append(wc)
                # a1 as a row (pattern A), then transpose for the p matvec
                pa1 = psA.tile([1, 512], F32, tag="ps", name="pa1")
                for k in range(8):
                    a, j = divmod(k, 4)
                    nc.tensor.matmul(pa1[:, :], ht[:, k:k + 1],
                                     attc[a][:, 512 * j:512 * (j + 1)],
                                     start=(k == 0), stop=(k == 7))
                a1b = sp.tile([1, 512], F32, tag="a1b")
                nc.vector.tensor_tensor(a1b[:], pa1[:, :], abr[:], ALU.add)
                a1r = sp.tile([1, 512], F32, tag="a1r")
                nc.scalar.activation(a1r[:], a1b[:], AF.Tanh)
                a1cp = psA.tile([128, 4], F32, tag="ps", name="a1cp")
                for m in range(4):
                    nc.tensor.matmul(a1cp[:, m:m + 1],
                                     a1r[0:1, 128 * m:128 * (m + 1)],
                                     onr[0:1, 0:1], start=True, stop=True)
                a1s = sp.tile([128, 4], F32, tag="a1s")
                nc.vector.tensor_copy(a1s[:], a1cp[:, :])
                # p = S * sigmoid(a1 @ w2 + b2)
                pp = psA.tile([1, 1], F32, tag="ps")
                for m in range(4):
                    nc.tensor.matmul(pp[:, :], a1s[:, m:m + 1], aw2[:, m:m + 1],
                                     start=(m == 0), stop=(m == 3))
                psig = sp.tile([1, 1], F32, tag="psig")
                nc.scalar.activation(psig[:], pp[:, :], AF.Sigmoid, bias=ab2[0:1, 0:1])
                pv = sp.tile([1, 1], F32, tag="pv")
                nc.vector.tensor_scalar(pv[:], psig[:], scs[0:1, 0:1], None, ALU.mult)

                # wr = [we, ws, hi, p]  (rounded window bounds)
                wr = sp.tile([1, 4], F32, tag="wr")
                tA = sp.tile([1, 2], F32, tag="tA")
                tB = sp.tile([1, 2], F32, tag="tB")
                # raw ws -> tA[0:1], raw we -> tA[1:2]
                nc.vector.tensor_scalar(tA[:, 0:1], pv[:], 64.0, 0.0, ALU.subtract, ALU.max)
                nc.vector.tensor_scalar(tA[:, 1:2], pv[:], 64.0, scs[0:1, 1:2], ALU.add, ALU.min)
                # round to nearest (even): f32 -> int32 -> f32 convert
                ti = sp.tile([1, 2], mybir.dt.int32, tag="ti")
                nc.vector.tensor_copy(ti[:], tA[:, :])
                nc.vector.tensor_copy(tB[:, :], ti[:])
                # wr[1] = ws, wr[0] = we
                nc.vector.tensor_copy(wr[:, 1:2], tB[:, 0:1])
                nc.vector.tensor_copy(wr[:, 0:1], tB[:, 1:2])
                # hi = min(we, ws + 128)
                hi_t = sp.tile([1, 1], F32, tag="hi_t")
                nc.vector.tensor_scalar(hi_t[:], wr[:, 1:2], 128.0, None, ALU.add)
                nc.vector.tensor_tensor(wr[:, 2:3], hi_t[:], wr[:, 0:1], ALU.min)
                # wr[3] = -p/2048 (gauss exponent bias)
                nc.vector.tensor_scalar(wr[:, 3:4], pv[:], float(-1.0 / STD2), None, ALU.mult)
                nc.scalar.dma_start(aux[0:1, :], wr[:])
                # broadcast [ws, hi, -p/2048] to all partitions via K=1 matmul
                wbp = psA.tile([128, 3], F32, tag="ps")
                nc.tensor.matmul(wbp[:, :], onr[:], wr[:, 1:4], start=True, stop=True)
                wb = sp.tile([128, 3], F32, tag="wb")
                nc.vector.tensor_copy(wb[:], wbp[:, :])

                # scores (pattern B over encT shard), m outer for group safety
                encc = []
                for a in range(2):
                    ec = encTp.tile([128, 2048], F32, tag="encT", name=f"encT{a}")
                    nc.sync.dma_start(
                        ec[:].rearrange("p (j n) -> p j n", j=4),
                        encT[512 * a:512 * (a + 1), :].rearrange("(j p) n -> p j n", p=128))
                    encc.append(ec)
                psr = psA.tile([1, 512], F32, tag="ps", name="psr")
                for k in range(8):
                    a, j = divmod(k, 4)
                    nc.tensor.matmul(psr[:, :], ht[:, k:k + 1],
                                     encc[a][:, 512 * j:512 * (j + 1)],
                                     start=(k == 0), stop=(k == 7))
                scr = sp.tile([1, 512], F32, tag="scr")
                nc.vector.tensor_copy(scr[:], psr[:, :])
                sp4 = psA.tile([128, 4], F32, tag="ps", name="sp4")
                for m in range(4):
                    nc.tensor.matmul(sp4[:, m:m + 1],
                                     scr[0:1, 128 * m:128 * (m + 1)],
                                     onr[0:1, 0:1], start=True, stop=True)
                # masked exp, gauss weights
                tge = sp.tile([128, 4], F32, tag="tge")
                nc.vector.tensor_scalar(tge[:], io4[:], wb[:, 0:1], None, ALU.is_ge)
                tle = sp.tile([128, 4], F32, tag="tle")
                nc.vector.tensor_scalar(tle[:], io4[:], wb[:, 1:2], None, ALU.is_le)
                tmask = sp.tile([128, 4], F32, tag="tmask")
                nc.vector.tensor_tensor(tmask[:], tge[:], tle[:], ALU.mult)
                e4 = sp.tile([128, 4], F32, tag="e4")
                nc.scalar.activation(e4[:], sp4[:, :], AF.Exp)
                em = sp.tile([128, 4], F32, tag="em")
                nc.vector.tensor_tensor(em[:], e4[:], tmask[:], ALU.mult)
                zr = sp.tile([128, 1], F32, tag="zr")
                nc.vector.tensor_reduce(zr[:], em[:], mybir.AxisListType.X, ALU.add)
                # gauss = exp(iota/2048 - p/2048), bias from broadcast col
                gex = sp.tile([128, 4], F32, tag="gex")
                nc.scalar.activation(gex[:], io4s[:], AF.Exp, bias=wb[:, 2:3])
                w4 = sp.tile([128, 4], F32, tag="w4")
                nc.vector.tensor_tensor(w4[:], em[:], gex[:], ALU.mult)
                zp = psA.tile([1, 1], F32, tag="ps")
                nc.tensor.matmul(zp[:, :], zr[:], onc[:], start=True, stop=True)
                zq = sp.tile([1, 1], F32, tag="zq")
                nc.vector.tensor_copy(zq[:], zp[:, :])
                # ctx partial (pattern A over encN shard), n outer
                encn = []
                for a in range(2):
                    en = encNp.tile([128, 2048], F32, tag="encN", name=f"encN{a}")
                    di = nc.sync.dma_start(
                        en[:].rearrange("p (j n) -> p j n", j=2),
                        encN[256 * a:256 * (a + 1), :].rearrange("(j p) n -> p j n", p=128))
                    bass._add_dep_helper(di.ins, agmark[0].ins, sync=True,
                                         reason="stage encN after AG1")
                    encn.append(en)
                ctxp = psA.tile([1, 1024], F32, tag="ps")
                for n in range(2):
                    for k in range(4):
                        a, j = divmod(k, 2)
                        nc.tensor.matmul(
                            ctxp[:, 512 * n:512 * (n + 1)],
                            w4[:, k:k + 1],
                            encn[a][:, 1024 * j + 512 * n:1024 * j + 512 * (n + 1)],
                            start=(k == 0), stop=(k == 3))
                ctxs = sp.tile([1, 1024], F32, tag="ctxs")
                nc.vector.tensor_copy(ctxs[:], ctxp[:, :])
                nc.scalar.dma_start(ar_in[0:1024, :], ctxs[:])
                nc.scalar.dma_start(ar_in[1024:1025, :], zq[:])
                nc.scalar.dma_start(ar_in[1025:1032, :], zpd[:, 0:7])
                nc.gpsimd.collective_compute(
                    "AllReduce", ALU.add, replica_groups=RG,
                    ins=[ar_in.opt()], outs=[ar_out.opt()])
                # 1/Z and normalized outputs
                zt = sp.tile([1, 1], F32, tag="zt")
                nc.scalar.dma_start(zt[:], ar_out[1024:1025, :])
                rz = sp.tile([1, 1], F32, tag="rz")
                nc.vector.reciprocal(rz[:], zt[:])
                rzp = psA.tile([128, 1], F32, tag="ps")
                nc.tensor.matmul(rzp[:, :], onr[:], rz[:], start=True, stop=True)
                rb = sp.tile([128, 1], F32, tag="rb")
                nc.vector.tensor_copy(rb[:], rzp[:, :])
                a4 = sp.tile([128, 4], F32, tag="a4")
                nc.vector.tensor_scalar(a4[:], w4[:], rb[:, 0:1], None, ALU.mult)
                nc.scalar.dma_start(a_part[:, :], a4[:])
                ctxrow = sp.tile([1, 1024], F32, tag="ctxrow")
                nc.scalar.dma_start(
                    ctxrow[:],
                    ar_out[0:1024, :].rearrange("(o p) q -> o (p q)", o=1))
                ccp = psA.tile([128, 8], F32, tag="ps", name="ccp")
                for t in range(8):
                    nc.tensor.matmul(ccp[:, t:t + 1],
                                     ctxrow[0:1, 128 * t:128 * (t + 1)],
                                     onr[0:1, 0:1], start=True, stop=True)
                ctxn = sp.tile([128, 8], F32, tag="ctxn")
                nc.vector.tensor_scalar(ctxn[:], ccp[:, :], rb[:, 0:1], None, ALU.mult)

                # ================= fc1 ===================================
                op_ = psA.tile([128, 1], F32, tag="ps")
                for a in range(2):
                    fct = fc1p.tile([128, 1024], F32, tag="fc1w")
                    di = nc.sync.dma_start(
                        fct[:].rearrange("p (j n) -> p j n", j=8),
                        fc1w[1024 * a:1024 * (a + 1), :].rearrange("(j p) n -> p j n", p=128))
                    bass._add_dep_helper(di.ins, agmark[0].ins, sync=True,
                                         reason="stage fc1w after AG1")
                    for j in range(8):
                        k = 8 * a + j
                        z = ctxn[:, k:k + 1] if k < 8 else ht[:, k - 8:k - 7]
                        nc.tensor.matmul(op_[:, :], fct[:, 128 * j:128 * (j + 1)], z,
                                         start=(k == 0), stop=(k == 15))
                oc = sp.tile([128, 1], F32, tag="oc")
                nc.scalar.activation(oc[:], op_[:, :], AF.Tanh, bias=f1b[:, 0:1])
                nc.scalar.dma_start(out_part[:, :], oc[:])
                ocb = sp.tile([128, 1], BF16, tag="ocb")
                nc.vector.tensor_copy(ocb[:], oc[:])

            # ================= fc2 + log_softmax =========================
            # fc2 is sharded over the CONTRACTION (each core owns 128 rows
            # of `out` and the matching 128 rows of fc2_w.T for the FULL
            # padded vocab). Partial logits live as (128, 256) columns,
            # summed across cores with one AllReduce; log-softmax then
            # runs fully local on every core.
            NT = VP // 128   # 256 column tiles
            with tc.tile_pool(name="psB", bufs=1, space="PSUM") as psB:
                fkc = []
                for ch in range(8):
                    fk = fc2p.tile([128, NT // 8 * 128], BF16, tag="fc2w_w",
                                   name=f"fc2w{ch}", bufs=6 if include_hh else 8)
                    di = nc.sync.dma_start(fk[:], fc2w[:, 4096 * ch:4096 * (ch + 1)])
                    bass._add_dep_helper(di.ins, agmark[0].ins, sync=True,
                                         reason="stage fc2w after AR_A")
                    fkc.append(fk)
                Lp = psB.tile([128, NT], F32, tag="L")
                for n in range(NT):
                    ch, o = divmod(n, 32)
                    nc.tensor.matmul(Lp[:, n:n + 1],
                                     fkc[ch][:, 128 * o:128 * (o + 1)],
                                     ocb[:, 0:1], start=True, stop=True)
                Ls = sp.tile([128, NT], BF16, tag="Ls")
                nc.vector.tensor_copy(Ls[:], Lp[:, :])
                nc.scalar.dma_start(arl_in[:, :], Ls[:])
                nc.gpsimd.collective_compute(
                    "AllReduce", ALU.add, replica_groups=RG,
                    ins=[arl_in.opt()], outs=[arl_out.opt()])
                La = sp.tile([128, NT], BF16, tag="La")
                nc.scalar.dma_start(La[:], arl_out[:, :])
                bcols = sp.tile([128, NT], F32, tag="bcols")
                nc.sync.dma_start(bcols[:], fc2b[:, :])
                Lb = sp.tile([128, NT], F32, tag="Lb")
                nc.vector.tensor_tensor(Lb[:], La[:], bcols[:], ALU.add)
                ex = sp.tile([128, NT], F32, tag="ex")
                zcol = sp.tile([128, 1], F32, tag="zcol")
                nc.scalar.activation(ex[:], Lb[:], AF.Exp, accum_out=zcol[:])
                zps = psB.tile([1, 1], F32, tag="zps", name="zps")
                nc.tensor.matmul(zps[:, :], zcol[:], onc[:], start=True, stop=True)
                lgz = sp.tile([1, 1], F32, tag="lgz")
                nc.scalar.activation(lgz[:], zps[:, :], AF.Ln)
                lgb = psB.tile([128, 1], F32, tag="lgb", name="lgb")
                nc.tensor.matmul(lgb[:, :], onr[:], lgz[:], start=True, stop=True)
                lgs = sp.tile([128, 1], F32, tag="lgs")
                nc.vector.tensor_copy(lgs[:], lgb[:, :])
                ys = sp.tile([128, NT], F32, tag="ys")
                nc.vector.tensor_scalar(ys[:], Lb[:], lgs[:, 0:1], None, ALU.subtract)
                nc.scalar.dma_start(y_part[:, :], ys[:, :])

    _split_sync_waits(nc)
    return nc


_NC_CACHE = {}


def _get_nc(include_hh: bool):
    if include_hh not in _NC_CACHE:
        _NC_CACHE[include_hh] = _build(include_hh)
    return _NC_CACHE[include_hh]


# --------------------------------------------------------------------------
def _host_prep(inputs, include_hh):
    emb = np.asarray(inputs["embedding"], np.float32)
    word = int(np.asarray(inputs["word"]).reshape(-1)[0])
    x = emb[word]
    Sf = float(np.asarray(inputs["source_sentence_length"]))
    h0 = np.asarray(inputs["h0"], np.float32)
    c0 = np.asarray(inputs["c0"], np.float32)
    w_ih = np.asarray(inputs["lstm_w_ih"], np.float32)
    w_hh = np.asarray(inputs["lstm_w_hh"], np.float32)
    b_ih = np.asarray(inputs["lstm_b_ih"], np.float32)
    b_hh = np.asarray(inputs["lstm_b_hh"], np.float32)
    enc = np.ascontiguousarray(np.asarray(inputs["encoder_output"], np.float32)[:, 0, :])
    att1w = np.asarray(inputs["att_fc1_w"], np.float32)
    att1b = np.asarray(inputs["att_fc1_b"], np.float32)
    att2w = np.asarray(inputs["att_fc2_w"], np.float32)
    att2b = np.asarray(inputs["att_fc2_b"], np.float32)
    fc1w = np.asarray(inputs["fc1_w"], np.float32)
    fc1b = np.asarray(inputs["fc1_b"], np.float32)
    fc2w = np.asarray(inputs["fc2_w"], np.float32)
    fc2b = np.asarray(inputs["fc2_b"], np.float32)

    fc2w_p = np.zeros((VP, H), np.float32)
    fc2w_p[:V] = fc2w
    fc2b_p = np.full((VP,), NEG, np.float32)
    fc2b_p[:V] = fc2b
    encTf = np.ascontiguousarray(enc.T)
    attwT = np.ascontiguousarray(att1w.T)
    fc1wT = np.ascontiguousarray(fc1w.T)

    in_maps = []
    for c in range(NCORE):
        u = slice(128 * c, 128 * (c + 1))
        gr = np.concatenate([np.arange(128 * c, 128 * (c + 1)) + 1024 * g for g in range(4)])
        d = {}
        d["xcols"] = np.ascontiguousarray(x.reshape(8, 128).T)
        if include_hh:
            d["h0cols"] = np.ascontiguousarray(
                np.concatenate([h0[l, 0].reshape(8, 128).T for l in range(L)], axis=1))
        d["c0row"] = np.ascontiguousarray(c0[0, 0, u].reshape(1, 128))
        d["c02c"] = np.ascontiguousarray(c0[1, 0].reshape(8, 128).T)  # (128,8)
        if include_hh:
            wc0 = np.concatenate([w_ih[0][gr].T, w_hh[0][gr].T])
        else:
            wc0 = w_ih[0][gr].T
        d["wcat0"] = np.ascontiguousarray(wc0)
        # layer 2: contraction-sharded -> own 128 h-rows of W2^T, all gates
        if include_hh:
            wc1 = np.concatenate([w_ih[1].T[u, :], w_hh[1].T[u, :]])  # (256, 4096)
        else:
            wc1 = w_ih[1].T[u, :]                                      # (128, 4096)
        d["wcat1"] = np.ascontiguousarray(wc1)
        d["biasg"] = np.ascontiguousarray(
            np.concatenate([b_ih[0][gr], b_hh[0][gr]]).reshape(1, 1024))
        d["bias2i"] = np.ascontiguousarray(b_ih[1].reshape(32, 128).T)
        d["bias2h"] = np.ascontiguousarray(b_hh[1].reshape(32, 128).T)
        d["oneh"] = np.zeros((1, 8), np.float32)
        d["oneh"][0, c] = 1.0
        if include_hh:
            d["h0sh"] = np.ascontiguousarray(h0[1, 0, u].reshape(128, 1))
        d["attw"] = attwT
        d["attb_col"] = np.ascontiguousarray(att1b.reshape(4, 128).T)
        d["attbr"] = np.ascontiguousarray(att1b.reshape(1, 512))
        d["attw2col"] = np.ascontiguousarray(att2w.reshape(-1).reshape(4, 128).T)
        d["attb2"] = np.ascontiguousarray(att2b.reshape(1, 1))
        d["sconst"] = np.array([[Sf, Sf - 1.0]], np.float32)
        d["iota4"] = np.ascontiguousarray(
            (512 * c + np.arange(512, dtype=np.float32)).reshape(4, 128).T)
        d["iotas"] = np.ascontiguousarray(d["iota4"] / np.float32(2048.0))
        d["encT"] = np.ascontiguousarray(encTf[:, 512 * c:512 * (c + 1)])
        d["encN"] = np.ascontiguousarray(enc[512 * c:512 * (c + 1)])
        d["fc1w"] = np.ascontiguousarray(fc1wT[:, u])
        d["fc1b_col"] = np.ascontiguousarray(fc1b[u].reshape(128, 1))
        d["fc2w"] = np.ascontiguousarray(fc2w_p[:, u].T).astype(bfloat16)
        d["fc2b"] = np.ascontiguousarray(fc2b_p.reshape(VP // 128, 128).T)
        d["onesr"] = np.ones((1, 128), np.float32)
        d["onesc"] = np.ones((128, 1), np.float32)
        d["zpad"] = np.zeros((1, 8), np.float32)
        in_maps.append(d)
    return in_maps


def _unshard(results, inputs):
    y = results[0]["y_part"].T.reshape(-1)[:V]
    out_vec = np.concatenate([results[c]["out_part"][:, 0] for c in range(NCORE)])
    blk = results[0]["hc1blk"].reshape(8, 256)
    h1 = blk[:, 0:128].reshape(-1)
    c1 = blk[:, 128:256].reshape(-1)
    h2 = results[0]["h2cols"].T.reshape(-1)
    c2 = results[0]["c2cols"].T.reshape(-1)
    h_n = np.stack([h1, h2])
    c_n = np.stack([c1, c2])
    a_full = np.concatenate([results[c]["a_part"].T.reshape(-1) for c in range(NCORE)])
    we = int(results[0]["aux"][0, 0])
    ws = int(results[0]["aux"][0, 1])
    Sv = int(np.asarray(inputs["source_sentence_length"]))
    idx = ws + np.arange(WIN)
    valid = idx <= we
    a = np.where(valid, a_full[np.clip(idx, 0, Sv - 1)], 0.0).astype(np.float32)
    return (
        y.reshape(1, 1, V).astype(np.float32),
        out_vec.reshape(1, 1, H).astype(np.float32),
        h_n[:, None, :].astype(np.float32),
        c_n[:, None, :].astype(np.float32),
        a.reshape(1, 1, WIN),
    )


def kernel(**inputs):
    h0 = np.asarray(inputs["h0"])
    include_hh = bool(np.any(h0 != 0))
    nc = _get_nc(include_hh)
    in_maps = _host_prep(inputs, include_hh)
    res = run_bass_kernel_spmd(nc, in_maps, core_ids=list(range(NCORE)))
    return _unshard(res.results, inputs)
